# revision 12
# baseline (speedup 1.0000x reference)
"""FAConv + LayerNorm + ReLU fused Trainium2 kernel (8 NeuronCores, SPMD).

Strategy (v3):
  Host: sort edges by destination 128-node block (core k owns 49 blocks =
  a contiguous 6272-node output shard -> no all-reduce), split each block's
  edges by src < 25088 (int16 gather-index limit) and by dst_local < 64
  (one-hot window), pack per (block, half) as [a x W64@0][1 x W128]
  [b x W64@64] tiles of 128 edges (straddle tile absorbs bin remainders ->
  ~zero SPMD padding), edges sorted by src within regions for HBM gather
  locality.
  Phase A (data-parallel): whole-shard SBUF residency; a_l/a_r = node @ att
  via broadcast DVE multiply + reductions; bf16 node table cast on Scalar;
  3 big DMAs total.
  Host: concat shards; permute a_l by edge src and a_r by edge dst into the
  padded tile layout as float16 streams (data movement only).
  Phase B (edge-parallel): coef = tanh(a_l+a_r)*w whole-array; one-hot stat
  tiles built 64-wide (or 128-wide for straddles) with stride-0 broadcast
  APs; src rows dma_gathered in 1024-row calls; segment-sum as
  PSUM-accumulated matmuls writing 64-partition windows (0.1*I identity
  matmul folds the eps*node_0 skip and zeroes PSUM); LayerNorm stats
  accumulated per block by 2 Scalar ops during PSUM drain; normalization +
  ReLU whole-array at the end.
"""
import sys

for _p in ('/opt/trn_rl_repo', '/root/.axon_site/_ro/trn_rl_repo'):
    if _p not in sys.path:
        sys.path.insert(0, _p)

import numpy as np
import ml_dtypes

import concourse.bass as bass
import concourse.bacc as bacc
import concourse.tile as tile
from concourse import mybir
from concourse.bass_utils import run_bass_kernel_spmd

N = 50000
D = 256
NCORES = 8
BPC = 49                    # dst blocks per core
NPAD = NCORES * BPC * 128   # 50176
NSH = BPC * 128             # 6272 nodes per core shard
HALF = NPAD // 2            # 25088 (int16-safe gather index range)
EPS_FA = 0.1
EPS_LN = 1e-5
MAXG = 8                    # tiles (of 128 idxs) per dma_gather call
C64 = 32                    # W64 tiles per batched one-hot build
C128 = 16                   # straddle tiles per batched build

f32 = mybir.dt.float32
bf16 = mybir.dt.bfloat16
f16 = mybir.dt.float16
i16 = mybir.dt.int16
AF = mybir.ActivationFunctionType
OP = mybir.AluOpType

_cache = {}


def _shard_ap(t):
    return bass.AP(t, 0, [(D, 128), (128 * D, BPC), (1, D)])


def _build_phase_a():
    nc = bacc.Bacc("TRN2", target_bir_lowering=False, debug=False,
                   num_devices=NCORES)
    node_sh = nc.declare_dram_parameter("node_sh", [NSH, D], f32, isOutput=False)
    att = nc.declare_dram_parameter("att", [2, D], f32, isOutput=False)
    aug_sh = nc.declare_dram_parameter("aug_sh", [NSH, D], f16, isOutput=True)
    alr_sh = nc.declare_dram_parameter("alr_sh", [128, 2 * BPC], f16, isOutput=True)

    with tile.TileContext(nc) as tc:
        with (
            tc.tile_pool(name="const", bufs=1) as cpool,
            tc.tile_pool(name="big", bufs=1) as big,
            tc.tile_pool(name="psum", bufs=2, space="PSUM") as psum,
        ):
            ones = cpool.tile([1, 128], f32)
            nc.vector.memset(ones[:], 1.0)
            att_bc = []
            for j in range(2):
                att_row = cpool.tile([1, D], f32, tag=f"attrow{j}")
                nc.sync.dma_start(out=att_row[:], in_=att[j:j + 1, :])
                ps = psum.tile([128, D], f32, tag="attps")
                nc.tensor.matmul(out=ps[:], lhsT=ones[:], rhs=att_row[:],
                                 start=True, stop=True)
                bc = cpool.tile([128, 1, D], f16, tag=f"attbc{j}")
                nc.vector.tensor_copy(bc[:, 0, :], ps[:])
                att_bc.append(bc)

            NCHUNK = 6
            bounds = [round(BPC * i / NCHUNK) for i in range(NCHUNK + 1)]
            alr = big.tile([128, 2 * BPC], f16, tag="alr")
            for ci in range(NCHUNK):
                b0, b1 = bounds[ci], bounds[ci + 1]
                nb = b1 - b0
                node_c = big.tile([128, nb, D], f32, tag=f"node{ci}")
                nc.sync.dma_start(
                    out=node_c[:],
                    in_=bass.AP(node_sh, b0 * 128 * D,
                                [(D, 128), (128 * D, nb), (1, D)]))
                aug_c = big.tile([128, nb, D], f16, tag=f"aug{ci}")
                nc.scalar.activation(out=aug_c[:], in_=node_c[:], func=AF.Copy)
                nc.sync.dma_start(
                    out=bass.AP(aug_sh, b0 * 128 * D,
                                [(D, 128), (128 * D, nb), (1, D)]),
                    in_=aug_c[:])
                # both dot products from the f16 table: mult (2-byte, fast
                # DVE mode eligible) + reduce, a_l on DVE and a_r on Pool
                for j, c0 in ((0, 0), (1, BPC)):
                    scr = big.tile([128, nb, D], f16, tag=f"scr{ci}_{j}")
                    # mults on Pool, reduces on DVE (Pool lacks X-reduce)
                    nc.gpsimd.tensor_tensor(
                        out=scr[:], in0=aug_c[:],
                        in1=att_bc[j][:].to_broadcast([128, nb, D]),
                        op=OP.mult)
                    with nc.allow_low_precision(
                            reason="f16 store of O(1) dot products; "
                                   "reduce accumulates in f32"):
                        nc.vector.tensor_reduce(
                            out=alr[:, c0 + b0:c0 + b1], in_=scr[:],
                            axis=mybir.AxisListType.X, op=OP.add)
            nc.sync.dma_start(out=alr_sh[:, :], in_=alr[:])
    nc.finalize()
    return nc


def _tile_counts(A, B):
    TL = int((A[:, 0] + B[:, 0]).sum()) + BPC
    TH = int((A[:, 1] + B[:, 1]).sum()) + BPC
    T64 = int((A + B).sum())
    T128 = 2 * BPC
    return TL, TH, T64, T128


def _build_phase_b(A, B, gb_identity):
    TL, TH, T64, T128 = _tile_counts(A, B)
    TT = T64 + T128
    nc = bacc.Bacc("TRN2", target_bir_lowering=False, debug=False,
                   num_devices=NCORES, num_swdge_queues=4)
    aug = nc.declare_dram_parameter("aug", [NPAD, D], f16, isOutput=False)
    idx_lo = nc.declare_dram_parameter("idx_lo", [128, max(8 * TL, 8)], i16,
                                       isOutput=False)
    idx_hi = nc.declare_dram_parameter("idx_hi", [128, max(8 * TH, 8)], i16,
                                       isOutput=False)
    dstl = nc.declare_dram_parameter("dstl", [128, TT], f16, isOutput=False)
    wgt = nc.declare_dram_parameter("wgt", [128, TT], f16, isOutput=False)
    alv = nc.declare_dram_parameter("alv", [128, TT], f16, isOutput=False)
    arv = nc.declare_dram_parameter("arv", [128, TT], f16, isOutput=False)
    node0_sh = nc.declare_dram_parameter("node0_sh", [NSH, D], f16,
                                         isOutput=False)
    gb = nc.declare_dram_parameter("gb", [1, 2 * D], f32, isOutput=False)
    iota64_in = nc.declare_dram_parameter("iota64_in", [128, 64], f16,
                                          isOutput=False)
    iota128_in = nc.declare_dram_parameter("iota128_in", [128, 128], f16,
                                           isOutput=False)
    epsi_in = nc.declare_dram_parameter("epsi_in", [128, 128], f16,
                                        isOutput=False)
    out_sh = nc.declare_dram_parameter("out_sh", [NSH, D], f16, isOutput=True)

    with tile.TileContext(nc) as tc:
        with (
            tc.tile_pool(name="const", bufs=1) as cpool,
            tc.tile_pool(name="big", bufs=1) as big,
            tc.tile_pool(name="glo", bufs=6) as glo,
            tc.tile_pool(name="ghi", bufs=6) as ghi,
            tc.tile_pool(name="eq64", bufs=2) as eqp64,
            tc.tile_pool(name="st64", bufs=2) as stp64,
            tc.tile_pool(name="eq128", bufs=2) as eqp128,
            tc.tile_pool(name="st128", bufs=2) as stp128,
            tc.tile_pool(name="epi", bufs=2) as epi,
            tc.tile_pool(name="psum", bufs=4, space="PSUM") as psum,
            tc.tile_pool(name="gbps", bufs=1, space="PSUM") as gbpsum,
        ):
            # ---- gather idx streams first: nothing else gates the gathers
            ilo = cpool.tile([128, max(8 * TL, 8)], i16, tag="ilo")
            nc.sync.dma_start(out=ilo[:], in_=idx_lo[:, :])
            ihi = cpool.tile([128, max(8 * TH, 8)], i16, tag="ihi")
            nc.sync.dma_start(out=ihi[:], in_=idx_hi[:, :])

            # ---- constants ----
            iota64 = cpool.tile([128, 1, 64], f16)
            nc.sync.dma_start(out=iota64[:, 0, :], in_=iota64_in[:, :])
            iota128 = cpool.tile([128, 1, 128], f16)
            nc.sync.dma_start(out=iota128[:, 0, :], in_=iota128_in[:, :])
            epsi = cpool.tile([128, 128], f16)
            nc.sync.dma_start(out=epsi[:], in_=epsi_in[:, :])
            if not gb_identity:
                ones_f = cpool.tile([1, 128], f32)
                nc.vector.memset(ones_f[:], 1.0)
                gb_row = cpool.tile([1, 2 * D], f32)
                nc.sync.dma_start(out=gb_row[:], in_=gb[:, :])
                gb_ps = gbpsum.tile([128, 2 * D], f32, tag="gbps")
                nc.tensor.matmul(out=gb_ps[:], lhsT=ones_f[:], rhs=gb_row[:],
                                 start=True, stop=True)
                gb_bc = cpool.tile([128, 2 * D], f32)
                nc.vector.tensor_copy(gb_bc[:], gb_ps[:])

            # ---- stream preload ----
            dstl_sb = cpool.tile([128, TT], f16, tag="dstl")
            nc.sync.dma_start(out=dstl_sb[:], in_=dstl[:, :])
            w_sb = cpool.tile([128, TT], f16, tag="w")
            nc.sync.dma_start(out=w_sb[:], in_=wgt[:, :])
            al_sb = cpool.tile([128, TT], f16, tag="al")
            nc.sync.dma_start(out=al_sb[:], in_=alv[:, :])
            ar_sb = cpool.tile([128, TT], f16, tag="ar")
            nc.sync.dma_start(out=ar_sb[:], in_=arv[:, :])
            node0_big = big.tile([128, BPC, D], f16, tag="node0")
            n0_bounds = [0, 4, 16, BPC]
            for b0, b1 in zip(n0_bounds[:-1], n0_bounds[1:]):
                nc.sync.dma_start(
                    out=node0_big[:, b0:b1, :],
                    in_=bass.AP(node0_sh, b0 * 128 * D,
                                [(D, 128), (128 * D, b1 - b0), (1, D)]))

            # ---- whole-array coef = tanh(al + ar) * w, cast bf16 ----
            arg_sb = cpool.tile([128, TT], f32, tag="arg")
            nc.vector.tensor_tensor(out=arg_sb[:], in0=al_sb[:], in1=ar_sb[:],
                                    op=OP.add)
            tanh16 = cpool.tile([128, TT], f16, tag="tanh16")
            nc.scalar.activation(out=tanh16[:], in_=arg_sb[:], func=AF.Tanh)
            coef_sb = cpool.tile([128, TT], f16, tag="coef")
            nc.vector.tensor_tensor(out=coef_sb[:], in0=tanh16[:],
                                    in1=w_sb[:], op=OP.mult)

            # ---- LN stat accumulators + x staging ----
            x_big = big.tile([128, BPC, D], f32, tag="x")
            sumx = big.tile([128, BPC], f32, tag="sumx")
            sumsq = big.tile([128, BPC], f32, tag="sumsq")
            sq_scr = epi.tile([128, D], f32, tag="sqscr")

            # ---- gather stream state (as v2) ----
            qctr = [0]

            class GS:
                def __init__(self, pool, isb, total):
                    self.pool, self.isb, self.total = pool, isb, total
                    self.col = 0
                    self.done = 0
                    self.gbt = None
                    self.slot = 0
                    self.cap = 0

            def next_tile(gs):
                if gs.gbt is None or gs.slot == gs.cap:
                    c = min(MAXG, gs.total - gs.done)
                    gs.gbt = gs.pool.tile([128, MAXG, D], f16, tag="g")
                    nc.gpsimd.dma_gather(
                        out_ap=gs.gbt[:, 0:c, :], in_ap=gs.base,
                        idxs_ap=gs.isb[:, gs.col:gs.col + 8 * c],
                        num_idxs=c * 128, num_idxs_reg=c * 128,
                        elem_size=D,
                        queue_num=qctr[0] % 4)
                    qctr[0] += 1
                    gs.col += 8 * c
                    gs.done += c
                    gs.slot, gs.cap = 0, c
                t = gs.gbt[:, gs.slot, :]
                gs.slot += 1
                return t

            lo = GS(glo, ilo, TL)
            lo.base = aug[0:HALF, :]
            hi = GS(ghi, ihi, TH)
            hi.base = aug[HALF:NPAD, :]

            # ---- one-hot stat builders (batched) ----
            class SB:
                def __init__(self, width, csz, eqp, stp, iota_bc, base, total):
                    self.width, self.csz = width, csz
                    self.eqp, self.stp, self.iota_bc = eqp, stp, iota_bc
                    self.base, self.total = base, total
                    self.done = 0
                    self.off = 0
                    self.cap = 0
                    self.cur = None

                def next(self):
                    if self.cur is None or self.off == self.cap:
                        c = min(self.csz, self.total - self.done)
                        col = self.base + self.done
                        eq = self.eqp.tile([128, self.csz, self.width], f16,
                                           tag="eq")
                        nc.vector.tensor_tensor(
                            out=eq[:, 0:c, :],
                            in0=self.iota_bc[:].to_broadcast(
                                [128, c, self.width]),
                            in1=dstl_sb[:, col:col + c].to_broadcast(
                                [128, c, self.width]),
                            op=OP.is_equal)
                        st = self.stp.tile([128, self.csz, self.width], f16,
                                           tag="st")
                        nc.vector.tensor_tensor(
                            out=st[:, 0:c, :],
                            in0=eq[:, 0:c, :],
                            in1=coef_sb[:, col:col + c].to_broadcast(
                                [128, c, self.width]),
                            op=OP.mult)
                        self.cur = st
                        self.off, self.cap = 0, c
                        self.done += c
                    t = self.cur[:, self.off, :]
                    self.off += 1
                    return t

            sb64 = SB(64, C64, eqp64, stp64, iota64, 0, T64)
            sb128 = SB(128, C128, eqp128, stp128, iota128, T64, T128)

            # ---- main loop ----
            ep_bounds = [0, 12, 24, 34, 42, 46, BPC]
            for i in range(BPC):
                acc = psum.tile([128, D], f32, tag="acc")
                nc.tensor.matmul(out=acc[:], lhsT=epsi[:],
                                 rhs=node0_big[:, i, :],
                                 start=True, stop=False,
                                 skip_group_check=True)
                seq = []
                for h in range(2):
                    a, b = int(A[i, h]), int(B[i, h])
                    seq += [(h, 0, 64)] * a + [(h, 0, 128)] + [(h, 64, 64)] * b
                for kt, (h, off, wdt) in enumerate(seq):
                    gs = lo if h == 0 else hi
                    g = next_tile(gs)
                    st = (sb64 if wdt == 64 else sb128).next()
                    if wdt == 128:
                        out_ap = acc[:]
                    else:
                        out_ap = acc[off:off + 64, :]
                    nc.tensor.matmul(out=out_ap, lhsT=st, rhs=g,
                                     start=False, stop=(kt == len(seq) - 1),
                                     skip_group_check=True)
                nc.scalar.activation(out=x_big[:, i, :], in_=acc[:],
                                     func=AF.Copy,
                                     accum_out=sumx[:, i:i + 1])
                nc.scalar.activation(out=sq_scr[:], in_=acc[:],
                                     func=AF.Square,
                                     accum_out=sumsq[:, i:i + 1])

                if i + 1 in ep_bounds:
                    g0 = ep_bounds[ep_bounds.index(i + 1) - 1]
                    g1 = i + 1
                    ng = g1 - g0
                    negmean = epi.tile([128, ng], f32, tag="negmean")
                    nc.scalar.activation(out=negmean[:],
                                         in_=sumx[:, g0:g1], func=AF.Copy,
                                         scale=-1.0 / D)
                    msq = epi.tile([128, ng], f32, tag="msq")
                    nc.vector.tensor_tensor(out=msq[:], in0=negmean[:],
                                            in1=negmean[:], op=OP.mult)
                    var = epi.tile([128, ng], f32, tag="var")
                    nc.scalar.activation(out=var[:], in_=sumsq[:, g0:g1],
                                         func=AF.Copy,
                                         scale=1.0 / D, bias=EPS_LN)
                    nc.vector.tensor_tensor(out=var[:], in0=var[:],
                                            in1=msq[:], op=OP.subtract)
                    std = epi.tile([128, ng], f32, tag="std")
                    nc.scalar.activation(out=std[:], in_=var[:],
                                         func=AF.Sqrt)
                    rstd = epi.tile([128, ng], f32, tag="rstd")
                    nc.vector.reciprocal(rstd[:], std[:])
                    nmr = epi.tile([128, ng], f32, tag="nmr")
                    nc.vector.tensor_tensor(out=nmr[:], in0=negmean[:],
                                            in1=rstd[:], op=OP.mult)
                    xg = x_big[:, g0:g1, :]
                    yg = epi.tile([128, 12, D], f16, tag="y")
                    if gb_identity:
                        for bi in range(ng):
                            nc.scalar.activation(
                                out=yg[:, bi, :],
                                in_=x_big[:, g0 + bi, :],
                                func=AF.Relu,
                                scale=rstd[:, bi:bi + 1],
                                bias=nmr[:, bi:bi + 1])
                    else:
                        nc.vector.tensor_tensor(
                            out=xg, in0=xg,
                            in1=negmean[:].to_broadcast([128, ng, D]),
                            op=OP.add)
                        nc.vector.tensor_tensor(
                            out=xg, in0=xg,
                            in1=rstd[:].to_broadcast([128, ng, D]),
                            op=OP.mult)
                        nc.vector.tensor_tensor(
                            out=xg, in0=xg,
                            in1=gb_bc[:, 0:D].to_broadcast([128, ng, D]),
                            op=OP.mult)
                        nc.vector.tensor_tensor(
                            out=xg, in0=xg,
                            in1=gb_bc[:, D:2 * D].to_broadcast([128, ng, D]),
                            op=OP.add)
                        nc.scalar.activation(out=yg[:, 0:ng, :], in_=xg,
                                             func=AF.Relu)
                    nc.sync.dma_start(
                        out=bass.AP(out_sh, g0 * 128 * D,
                                    [(D, 128), (128 * D, ng), (1, D)]),
                        in_=yg[:, 0:ng, :])
    nc.finalize()
    # Tile assigns DMASW sems round-robin (mod 8) over Pool DMA insts in
    # FINAL scheduled order; a sem must stay locked to one SWDGE queue ->
    # rewrite queue_num to final_order_idx % 4.
    gi = 0
    for bb in nc.m.functions[0].blocks:
        for inst in bb.instructions:
            if type(inst).__name__ == 'InstDMAGatherAnt':
                inst.queue_num = gi % 4
                gi += 1
    return nc


def _pack_gather_idxs(vals, total_tiles):
    ncols = 8 * int(total_tiles)
    arr = np.zeros((16, max(ncols, 8)), np.int16)
    v = np.zeros(int(total_tiles) * 128, np.int16)
    v[:len(vals)] = vals
    col = 0
    done = 0
    while done < total_tiles:
        c = int(min(MAXG, total_tiles - done))
        chunk = v[done * 128:(done + c) * 128]
        arr[:, col:col + 8 * c] = chunk.reshape(8 * c, 16).T
        col += 8 * c
        done += c
    return np.tile(arr, (8, 1))


def _slot_tiles(n0, n1):
    """Static straddle-layout tile counts for one (slot, half) given
    per-core bin counts n0 (dst_local<64), n1 (>=64): [a x W64@0]
    [1 x W128][b x W64@64]."""
    a = max(0, -(-int(n0.max()) // 128) - 1)
    r0 = np.clip(n0 - 128 * a, 0, None)
    b = int((-(-np.clip(r0 + n1 - 128, 0, None) // 128)).max())
    return a, b


def _host_shard(src, dst, w):
    blk = dst >> 7
    local = (dst & 127).astype(np.int64)
    NB = NCORES * BPC
    h_ = (src >= HALF).astype(np.int64)
    b_ = (local >= 64).astype(np.int64)
    cnt4 = np.zeros((NB, 2, 2), np.int64)
    np.add.at(cnt4, (blk, h_, b_), 1)
    block2core, block2slot = _assign_blocks(cnt4)
    key = (((block2core[blk] * BPC + block2slot[blk]) * 2 + h_) * 2 + b_)
    order = np.lexsort((src, key))
    src_s = src[order].astype(np.int32)
    dst_s = dst[order].astype(np.int32)
    local_s = (dst_s & 127).astype(np.int32)
    w_s = w[order]
    cnt = np.bincount(key[order], minlength=4 * NB)
    offs = np.concatenate([[0], np.cumsum(cnt)])
    # per (core, slot, half, bin) counts
    C = np.zeros((NCORES, BPC, 2, 2), np.int64)
    for bb in range(NB):
        C[block2core[bb], block2slot[bb]] = cnt4[bb]
    A = np.zeros((BPC, 2), np.int64)
    B = np.zeros((BPC, 2), np.int64)
    for s in range(BPC):
        for h in range(2):
            A[s, h], B[s, h] = _slot_tiles(C[:, s, h, 0], C[:, s, h, 1])
    return (block2core, block2slot, offs, src_s, dst_s, local_s, w_s, A, B)


def _assign_blocks(cnt4):
    """Partition the 392 dst blocks into 49 slot-groups of 8 (one per core;
    which core is arbitrary since the host reorders output blocks).
    Objective: total straddle-layout tiles. Initial: size-sorted strips of
    8; then local search with cross-group member swaps."""
    NB = NCORES * BPC
    tot = cnt4.sum(axis=(1, 2))
    order = np.argsort(-tot, kind="stable")
    groups = order.reshape(BPC, NCORES).copy()

    def gcost(members):
        t = 0
        for h in range(2):
            n0 = cnt4[members, h, 0]
            n1 = cnt4[members, h, 1]
            a = max(0, -(-int(n0.max()) // 128) - 1)
            r0 = np.clip(n0 - 128 * a, 0, None)
            b = int((-(-np.clip(r0 + n1 - 128, 0, None) // 128)).max())
            t += a + 1 + b
        return t

    cost = np.array([gcost(groups[g]) for g in range(BPC)])
    rng = np.random.default_rng(11)
    ga_ = rng.integers(0, BPC, 150000)
    gb_ = rng.integers(0, BPC, 150000)
    ia_ = rng.integers(0, NCORES, 150000)
    ib_ = rng.integers(0, NCORES, 150000)
    for ga, gb, ia, ib in zip(ga_, gb_, ia_, ib_):
        if ga == gb:
            continue
        old = cost[ga] + cost[gb]
        groups[ga, ia], groups[gb, ib] = groups[gb, ib], groups[ga, ia]
        ca, cb = gcost(groups[ga]), gcost(groups[gb])
        if ca + cb <= old:
            cost[ga], cost[gb] = ca, cb
        else:
            groups[ga, ia], groups[gb, ib] = groups[gb, ib], groups[ga, ia]
    block2core = np.empty(NB, np.int64)
    block2slot = np.empty(NB, np.int64)
    for g in range(BPC):
        for m in range(NCORES):
            block2core[groups[g, m]] = m
            block2slot[groups[g, m]] = g
    return block2core, block2slot


def _pads(m):
    """m scattered ascending int16 idxs for gather pad slots."""
    return np.sort((np.arange(m, dtype=np.int64) * 97 % HALF)).astype(np.int16)


def _build_in_b(node_0, aug_full, al_full, ar_full,
                block2core, block2slot, offs, src_s, dst_s, local_s, w_s,
                A, B, ln_weight, ln_bias):
    NB = NCORES * BPC
    TL, TH, T64, T128 = _tile_counts(A, B)
    TT = T64 + T128
    node0_pad = np.zeros((NPAD, D), np.float32)
    node0_pad[:N] = node_0
    gbv = np.concatenate([ln_weight, ln_bias])[None, :]
    iota64_np = np.tile(np.arange(64, dtype=np.float16)[None, :], (128, 1))
    iota128_np = np.tile(np.arange(128, dtype=np.float16)[None, :], (128, 1))
    epsi_np = (EPS_FA * np.eye(128, dtype=np.float32)).astype(np.float16)
    al16 = al_full.astype(np.float16)
    ar16 = ar_full.astype(np.float16)
    w16 = w_s.astype(np.float16)
    in_b = []
    for k in range(NCORES):
        lo_vals, hi_vals = [], []
        dstl_arr = np.zeros((128, TT), np.float16)
        w_arr = np.zeros((128, TT), np.float16)
        al_arr = np.zeros((128, TT), np.float16)
        ar_arr = np.zeros((128, TT), np.float16)
        col64 = 0
        col128 = T64

        def put_cols(col, ntile, sl_list, sent, dsub, h):
            """Fill ntile columns starting at col from edge slices sl_list
            (concatenated, in order). dsub subtracted from dst_local;
            sent = pad sentinel; h picks the gather base half."""
            idxs = np.concatenate([np.arange(s.start, s.stop)
                                   for s in sl_list]) if sl_list else \
                np.zeros(0, np.int64)
            nv = len(idxs)
            cap = ntile * 128
            dv = np.full(cap, sent, np.float16)
            dv[:nv] = local_s[idxs] - dsub
            wv = np.zeros(cap, np.float16)
            wv[:nv] = w16[idxs]
            av = np.zeros(cap, np.float16)
            av[:nv] = al16[src_s[idxs]]
            rv = np.zeros(cap, np.float16)
            rv[:nv] = ar16[dst_s[idxs]]
            dstl_arr[:, col:col + ntile] = dv.reshape(ntile, 128).T
            w_arr[:, col:col + ntile] = wv.reshape(ntile, 128).T
            al_arr[:, col:col + ntile] = av.reshape(ntile, 128).T
            ar_arr[:, col:col + ntile] = rv.reshape(ntile, 128).T
            iv = _pads(cap)
            iv[:nv] = (src_s[idxs] - HALF * h).astype(np.int16)
            return iv

        for i in range(BPC):
            for h in range(2):
                ki = ((k * BPC + i) * 2 + h) * 2
                s0, s1, s2 = offs[ki], offs[ki + 1], offs[ki + 2]
                n0, n1 = s1 - s0, s2 - s1
                a, b = int(A[i, h]), int(B[i, h])
                m0 = min(128 * a, n0)
                r0 = n0 - m0
                stk = min(128 - r0, n1)
                coll = lo_vals if h == 0 else hi_vals
                # W64 @0 tiles (bin0 head)
                iv = put_cols(col64, a, [slice(s0, s0 + m0)] if a else [],
                              100.0, 0, h)
                if a:
                    coll.append(iv)
                # straddle (bin0 tail + bin1 head)
                iv = put_cols(col128, 1,
                              [slice(s0 + m0, s1), slice(s1, s1 + stk)],
                              200.0, 0, h)
                coll.append(iv)
                # W64 @64 tiles (bin1 tail)
                iv = put_cols(col64 + a, b,
                              [slice(s1 + stk, s2)] if b else [], 100.0, 64,
                              h)
                if b:
                    coll.append(iv)
                col64 += a + b
                col128 += 1
        blocks_k = np.array([np.where((block2core == k) & (block2slot == i))[0][0]
                             for i in range(BPC)])
        node0_k = node0_pad.reshape(NB, 128, D)[blocks_k].reshape(NSH, D)
        in_b.append({
            "aug": aug_full,
            "idx_lo": _pack_gather_idxs(np.concatenate(lo_vals), TL),
            "idx_hi": _pack_gather_idxs(np.concatenate(hi_vals), TH),
            "dstl": dstl_arr,
            "wgt": w_arr,
            "alv": al_arr,
            "arv": ar_arr,
            "node0_sh": node0_k.astype(np.float16),
            "gb": gbv,
            "iota64_in": iota64_np,
            "iota128_in": iota128_np,
            "epsi_in": epsi_np,
        })
        _cache.setdefault("blocks_by_core", {})[k] = blocks_k
    return in_b


def kernel(node, node_0, edge_index, edge_attr, batch_ptr,
           att_l, att_r, ln_weight, ln_bias):
    node = np.asarray(node, np.float32)
    node_0 = np.asarray(node_0, np.float32)
    src = np.asarray(edge_index[0], np.int64)
    dst = np.asarray(edge_index[1], np.int64)
    w = np.asarray(edge_attr, np.float32)
    att_l = np.asarray(att_l, np.float32)
    att_r = np.asarray(att_r, np.float32)
    ln_weight = np.asarray(ln_weight, np.float32)
    ln_bias = np.asarray(ln_bias, np.float32)

    (block2core, block2slot, offs, src_s, dst_s, local_s, w_s,
     A, B) = _host_shard(src, dst, w)

    gb_identity = bool(np.all(ln_weight == 1.0) and np.all(ln_bias == 0.0))
    sig = (A.tobytes(), B.tobytes(), gb_identity)
    if "A" not in _cache:
        _cache["A"] = _build_phase_a()
    if ("B", sig) not in _cache:
        _cache[("B", sig)] = _build_phase_b(A, B, gb_identity)
    nc_a = _cache["A"]
    nc_b = _cache[("B", sig)]

    # ---- phase A ----
    node_pad = np.zeros((NPAD, D), np.float32)
    node_pad[:N] = node
    att = np.stack([att_l, att_r])
    in_a = [{"node_sh": node_pad[k * NSH:(k + 1) * NSH], "att": att}
            for k in range(NCORES)]
    res_a = run_bass_kernel_spmd(nc_a, in_a, list(range(NCORES)),
                                 **_cache.get("runkw", {}))
    aug_full = np.concatenate([res_a.results[k]["aug_sh"]
                               for k in range(NCORES)])
    al_full = np.concatenate(
        [res_a.results[k]["alr_sh"][:, 0:BPC].T.reshape(NSH)
         for k in range(NCORES)])
    ar_full = np.concatenate(
        [res_a.results[k]["alr_sh"][:, BPC:2 * BPC].T.reshape(NSH)
         for k in range(NCORES)])
    t_a = res_a.exec_time_ns

    # ---- phase B ----
    in_b = _build_in_b(node_0, aug_full, al_full, ar_full,
                       block2core, block2slot, offs, src_s, dst_s, local_s,
                       w_s, A, B, ln_weight, ln_bias)
    res_b = run_bass_kernel_spmd(nc_b, in_b, list(range(NCORES)),
                                 **_cache.get("runkw", {}))
    NB = NCORES * BPC
    out = np.empty((NB, 128, D), np.float32)
    for k in range(NCORES):
        out[_cache["blocks_by_core"][k]] = \
            res_b.results[k]["out_sh"].astype(np.float32).reshape(BPC, 128, D)
    out = out.reshape(NPAD, D)
    t_b = res_b.exec_time_ns
    _cache["t_a_ns"] = t_a
    _cache["t_b_ns"] = t_b
    _cache["res_a"] = res_a
    _cache["res_b"] = res_b
    if t_a is not None and t_b is not None:
        _cache["last_exec_ns"] = t_a + t_b
    return out[:N]


# revision 22
# speedup vs baseline: 1.0066x; 1.0066x over previous
"""FAConv + LayerNorm + ReLU fused Trainium2 kernel (8 NeuronCores, SPMD).

Strategy (v3):
  Host: sort edges by destination 128-node block (core k owns 49 blocks =
  a contiguous 6272-node output shard -> no all-reduce), split each block's
  edges by src < 25088 (int16 gather-index limit) and by dst_local < 64
  (one-hot window), pack per (block, half) as [a x W64@0][1 x W128]
  [b x W64@64] tiles of 128 edges (straddle tile absorbs bin remainders ->
  ~zero SPMD padding), edges sorted by src within regions for HBM gather
  locality.
  Phase A (data-parallel): whole-shard SBUF residency; a_l/a_r = node @ att
  via broadcast DVE multiply + reductions; bf16 node table cast on Scalar;
  3 big DMAs total.
  Host: concat shards; permute a_l by edge src and a_r by edge dst into the
  padded tile layout as float16 streams (data movement only).
  Phase B (edge-parallel): coef = tanh(a_l+a_r)*w whole-array; one-hot stat
  tiles built 64-wide (or 128-wide for straddles) with stride-0 broadcast
  APs; src rows dma_gathered in 1024-row calls; segment-sum as
  PSUM-accumulated matmuls writing 64-partition windows (0.1*I identity
  matmul folds the eps*node_0 skip and zeroes PSUM); LayerNorm stats
  accumulated per block by 2 Scalar ops during PSUM drain; normalization +
  ReLU whole-array at the end.
"""
import sys

for _p in ('/opt/trn_rl_repo', '/root/.axon_site/_ro/trn_rl_repo'):
    if _p not in sys.path:
        sys.path.insert(0, _p)

import numpy as np
import ml_dtypes

import concourse.bass as bass
import concourse.bacc as bacc
import concourse.tile as tile
from concourse import mybir
from concourse.bass_utils import run_bass_kernel_spmd

N = 50000
D = 256
NCORES = 8
BPC = 49                    # dst blocks per core
NPAD = NCORES * BPC * 128   # 50176
NSH = BPC * 128             # 6272 nodes per core shard
HALF = NPAD // 2            # 25088 (int16-safe gather index range)
EPS_FA = 0.1
EPS_LN = 1e-5
MAXG = 8                    # tiles (of 128 idxs) per dma_gather call
C64 = 32                    # W64 tiles per batched one-hot build
C128 = 16                   # straddle tiles per batched build

f32 = mybir.dt.float32
bf16 = mybir.dt.bfloat16
f16 = mybir.dt.float16
i16 = mybir.dt.int16
AF = mybir.ActivationFunctionType
OP = mybir.AluOpType

_cache = {}


def _shard_ap(t):
    return bass.AP(t, 0, [(D, 128), (128 * D, BPC), (1, D)])


def _build_phase_a():
    nc = bacc.Bacc("TRN2", target_bir_lowering=False, debug=False,
                   num_devices=NCORES)
    node_sh = nc.declare_dram_parameter("node_sh", [NSH, D], f32, isOutput=False)
    att = nc.declare_dram_parameter("att", [2, D], f32, isOutput=False)
    aug_sh = nc.declare_dram_parameter("aug_sh", [NSH, D], f16, isOutput=True)
    alr_sh = nc.declare_dram_parameter("alr_sh", [128, 2 * BPC], f16, isOutput=True)

    with tile.TileContext(nc) as tc:
        with (
            tc.tile_pool(name="const", bufs=1) as cpool,
            tc.tile_pool(name="big", bufs=1) as big,
            tc.tile_pool(name="psum", bufs=2, space="PSUM") as psum,
        ):
            ones = cpool.tile([1, 128], f32)
            nc.vector.memset(ones[:], 1.0)
            att_bc = []
            for j in range(2):
                att_row = cpool.tile([1, D], f32, tag=f"attrow{j}")
                nc.sync.dma_start(out=att_row[:], in_=att[j:j + 1, :])
                ps = psum.tile([128, D], f32, tag="attps")
                nc.tensor.matmul(out=ps[:], lhsT=ones[:], rhs=att_row[:],
                                 start=True, stop=True)
                bc = cpool.tile([128, 1, D], f16, tag=f"attbc{j}")
                nc.vector.tensor_copy(bc[:, 0, :], ps[:])
                att_bc.append(bc)

            NCHUNK = 8
            bounds = [round(BPC * i / NCHUNK) for i in range(NCHUNK + 1)]
            alr = big.tile([128, 2 * BPC], f16, tag="alr")
            for ci in range(NCHUNK):
                b0, b1 = bounds[ci], bounds[ci + 1]
                nb = b1 - b0
                node_c = big.tile([128, nb, D], f32, tag=f"node{ci}")
                nc.sync.dma_start(
                    out=node_c[:],
                    in_=bass.AP(node_sh, b0 * 128 * D,
                                [(D, 128), (128 * D, nb), (1, D)]))
                aug_c = big.tile([128, nb, D], f16, tag=f"aug{ci}")
                nc.scalar.activation(out=aug_c[:], in_=node_c[:], func=AF.Copy)
                nc.sync.dma_start(
                    out=bass.AP(aug_sh, b0 * 128 * D,
                                [(D, 128), (128 * D, nb), (1, D)]),
                    in_=aug_c[:])
                # a_l: DVE mult + X-reduce.  a_r: mult on Pool (its only
                # job); reduce split Scalar-accum / DVE by measured rates.
                scr_l = big.tile([128, nb, D], f16, tag=f"scrl{ci}")
                nc.vector.tensor_tensor(
                    out=scr_l[:], in0=aug_c[:],
                    in1=att_bc[0][:].to_broadcast([128, nb, D]),
                    op=OP.mult)
                scr_r = big.tile([128, nb, D], f16, tag=f"scrr{ci}")
                nc.gpsimd.tensor_tensor(
                    out=scr_r[:], in0=aug_c[:],
                    in1=att_bc[1][:].to_broadcast([128, nb, D]),
                    op=OP.mult)
                with nc.allow_low_precision(
                        reason="f16 store of O(1) dot products; "
                               "reduce accumulates in f32"):
                    nc.vector.tensor_reduce(
                        out=alr[:, b0:b1], in_=scr_l[:],
                        axis=mybir.AxisListType.X, op=OP.add)
                    if ci >= 5:
                        nc.vector.tensor_reduce(
                            out=alr[:, BPC + b0:BPC + b1], in_=scr_r[:],
                            axis=mybir.AxisListType.X, op=OP.add)
                if ci < 5:
                    with nc.allow_low_precision(
                            reason="f16 store of O(1) dot products; "
                                   "Act accumulator is f32"):
                        for bi in range(nb):
                            nc.scalar.activation(
                                out=scr_r[:, bi, :], in_=scr_r[:, bi, :],
                                func=AF.Copy,
                                accum_out=alr[:, BPC + b0 + bi:
                                              BPC + b0 + bi + 1])
            nc.sync.dma_start(out=alr_sh[:, :], in_=alr[:])
    nc.finalize()
    return nc


def _tile_counts(A, B):
    TL = int((A[:, 0] + B[:, 0]).sum()) + BPC
    TH = int((A[:, 1] + B[:, 1]).sum()) + BPC
    T64 = int((A + B).sum())
    T128 = 2 * BPC
    return TL, TH, T64, T128


def _build_phase_b(A, B, gb_identity):
    TL, TH, T64, T128 = _tile_counts(A, B)
    TT = T64 + T128
    nc = bacc.Bacc("TRN2", target_bir_lowering=False, debug=False,
                   num_devices=NCORES, num_swdge_queues=4)
    aug = nc.declare_dram_parameter("aug", [NPAD, D], f16, isOutput=False)
    idx_lo = nc.declare_dram_parameter("idx_lo", [128, max(8 * TL, 8)], i16,
                                       isOutput=False)
    idx_hi = nc.declare_dram_parameter("idx_hi", [128, max(8 * TH, 8)], i16,
                                       isOutput=False)
    dstl = nc.declare_dram_parameter("dstl", [128, TT], f16, isOutput=False)
    wgt = nc.declare_dram_parameter("wgt", [128, TT], f16, isOutput=False)
    alv = nc.declare_dram_parameter("alv", [128, TT], f16, isOutput=False)
    arv = nc.declare_dram_parameter("arv", [128, TT], f16, isOutput=False)
    node0_sh = nc.declare_dram_parameter("node0_sh", [NSH, D], f16,
                                         isOutput=False)
    gb = nc.declare_dram_parameter("gb", [1, 2 * D], f32, isOutput=False)
    iota64_in = nc.declare_dram_parameter("iota64_in", [128, 64], f16,
                                          isOutput=False)
    iota128_in = nc.declare_dram_parameter("iota128_in", [128, 128], f16,
                                           isOutput=False)
    epsi_in = nc.declare_dram_parameter("epsi_in", [128, 128], f16,
                                        isOutput=False)
    out_sh = nc.declare_dram_parameter("out_sh", [NSH, D], f16, isOutput=True)

    with tile.TileContext(nc) as tc:
        with (
            tc.tile_pool(name="const", bufs=1) as cpool,
            tc.tile_pool(name="big", bufs=1) as big,
            tc.tile_pool(name="glo", bufs=6) as glo,
            tc.tile_pool(name="ghi", bufs=6) as ghi,
            tc.tile_pool(name="eq64", bufs=2) as eqp64,
            tc.tile_pool(name="st64", bufs=2) as stp64,
            tc.tile_pool(name="eq128", bufs=2) as eqp128,
            tc.tile_pool(name="st128", bufs=2) as stp128,
            tc.tile_pool(name="epi", bufs=2) as epi,
            tc.tile_pool(name="psum", bufs=4, space="PSUM") as psum,
            tc.tile_pool(name="gbps", bufs=1, space="PSUM") as gbpsum,
        ):
            # ---- gather idx streams first: nothing else gates the gathers.
            # Issued from the (idle) Scalar/Vector engine DMA queues so they
            # don't serialize behind the Sync engine's preload queue.
            ilo = cpool.tile([128, max(8 * TL, 8)], i16, tag="ilo")
            nc.scalar.dma_start(out=ilo[:], in_=idx_lo[:, :])
            ihi = cpool.tile([128, max(8 * TH, 8)], i16, tag="ihi")
            nc.scalar.dma_start(out=ihi[:], in_=idx_hi[:, :])

            # ---- constants ----
            iota64 = cpool.tile([128, 1, 64], f16)
            nc.sync.dma_start(out=iota64[:, 0, :], in_=iota64_in[:, :])
            iota128 = cpool.tile([128, 1, 128], f16)
            nc.sync.dma_start(out=iota128[:, 0, :], in_=iota128_in[:, :])
            epsi = cpool.tile([128, 128], f16)
            nc.sync.dma_start(out=epsi[:], in_=epsi_in[:, :])
            if not gb_identity:
                ones_f = cpool.tile([1, 128], f32)
                nc.vector.memset(ones_f[:], 1.0)
                gb_row = cpool.tile([1, 2 * D], f32)
                nc.sync.dma_start(out=gb_row[:], in_=gb[:, :])
                gb_ps = gbpsum.tile([128, 2 * D], f32, tag="gbps")
                nc.tensor.matmul(out=gb_ps[:], lhsT=ones_f[:], rhs=gb_row[:],
                                 start=True, stop=True)
                gb_bc = cpool.tile([128, 2 * D], f32)
                nc.vector.tensor_copy(gb_bc[:], gb_ps[:])

            # ---- stream preload: priority head so the first stat chunks
            # and coef columns are ready early, remainder via Sync ----
            PRI = T128 + 2 * C64
            dstl_sb = cpool.tile([128, TT], f16, tag="dstl")
            nc.scalar.dma_start(out=dstl_sb[:, 0:PRI], in_=dstl[:, 0:PRI])
            nc.sync.dma_start(out=dstl_sb[:, PRI:TT], in_=dstl[:, PRI:TT])
            w_sb = cpool.tile([128, TT], f16, tag="w")
            nc.scalar.dma_start(out=w_sb[:, 0:PRI], in_=wgt[:, 0:PRI])
            nc.sync.dma_start(out=w_sb[:, PRI:TT], in_=wgt[:, PRI:TT])
            al_sb = cpool.tile([128, TT], f16, tag="al")
            nc.scalar.dma_start(out=al_sb[:, 0:PRI], in_=alv[:, 0:PRI])
            nc.sync.dma_start(out=al_sb[:, PRI:TT], in_=alv[:, PRI:TT])
            ar_sb = cpool.tile([128, TT], f16, tag="ar")
            nc.scalar.dma_start(out=ar_sb[:, 0:PRI], in_=arv[:, 0:PRI])
            nc.sync.dma_start(out=ar_sb[:, PRI:TT], in_=arv[:, PRI:TT])
            node0_big = big.tile([128, BPC, D], f16, tag="node0")
            n0_bounds = [0, 4, 16, BPC]
            for b0, b1 in zip(n0_bounds[:-1], n0_bounds[1:]):
                nc.sync.dma_start(
                    out=node0_big[:, b0:b1, :],
                    in_=bass.AP(node0_sh, b0 * 128 * D,
                                [(D, 128), (128 * D, b1 - b0), (1, D)]))

            # ---- whole-array coef = tanh(al + ar) * w, priority head first
            arg_sb = cpool.tile([128, TT], f32, tag="arg")
            tanh16 = cpool.tile([128, TT], f16, tag="tanh16")
            coef_sb = cpool.tile([128, TT], f16, tag="coef")
            for c0, c1 in ((0, PRI), (PRI, TT)):
                nc.vector.tensor_tensor(out=arg_sb[:, c0:c1],
                                        in0=al_sb[:, c0:c1],
                                        in1=ar_sb[:, c0:c1], op=OP.add)
                nc.scalar.activation(out=tanh16[:, c0:c1],
                                     in_=arg_sb[:, c0:c1], func=AF.Tanh)
                nc.vector.tensor_tensor(out=coef_sb[:, c0:c1],
                                        in0=tanh16[:, c0:c1],
                                        in1=w_sb[:, c0:c1], op=OP.mult)

            # ---- LN stat accumulators + x staging ----
            x_big = big.tile([128, BPC, D], f32, tag="x")
            sumx = big.tile([128, BPC], f32, tag="sumx")
            sumsq = big.tile([128, BPC], f32, tag="sumsq")
            sq_scr = epi.tile([128, D], f32, tag="sqscr")

            # ---- gather stream state (as v2) ----
            qctr = [0]

            class GS:
                def __init__(self, pool, isb, total):
                    self.pool, self.isb, self.total = pool, isb, total
                    self.col = 0
                    self.done = 0
                    self.gbt = None
                    self.slot = 0
                    self.cap = 0

            def next_tile(gs):
                if gs.gbt is None or gs.slot == gs.cap:
                    c = min(MAXG, gs.total - gs.done)
                    gs.gbt = gs.pool.tile([128, MAXG, D], f16, tag="g")
                    nc.gpsimd.dma_gather(
                        out_ap=gs.gbt[:, 0:c, :], in_ap=gs.base,
                        idxs_ap=gs.isb[:, gs.col:gs.col + 8 * c],
                        num_idxs=c * 128, num_idxs_reg=c * 128,
                        elem_size=D,
                        queue_num=qctr[0] % 4)
                    qctr[0] += 1
                    gs.col += 8 * c
                    gs.done += c
                    gs.slot, gs.cap = 0, c
                t = gs.gbt[:, gs.slot, :]
                gs.slot += 1
                return t

            lo = GS(glo, ilo, TL)
            lo.base = aug[0:HALF, :]
            hi = GS(ghi, ihi, TH)
            hi.base = aug[HALF:NPAD, :]

            # ---- one-hot stat builders (batched) ----
            class SB:
                def __init__(self, width, csz, eqp, stp, iota_bc, base, total):
                    self.width, self.csz = width, csz
                    self.eqp, self.stp, self.iota_bc = eqp, stp, iota_bc
                    self.base, self.total = base, total
                    self.done = 0
                    self.off = 0
                    self.cap = 0
                    self.cur = None

                def next(self):
                    if self.cur is None or self.off == self.cap:
                        c = min(self.csz, self.total - self.done)
                        col = self.base + self.done
                        eq = self.eqp.tile([128, self.csz, self.width], f16,
                                           tag="eq")
                        nc.vector.tensor_tensor(
                            out=eq[:, 0:c, :],
                            in0=self.iota_bc[:].to_broadcast(
                                [128, c, self.width]),
                            in1=dstl_sb[:, col:col + c].to_broadcast(
                                [128, c, self.width]),
                            op=OP.is_equal)
                        st = self.stp.tile([128, self.csz, self.width], f16,
                                           tag="st")
                        nc.vector.tensor_tensor(
                            out=st[:, 0:c, :],
                            in0=eq[:, 0:c, :],
                            in1=coef_sb[:, col:col + c].to_broadcast(
                                [128, c, self.width]),
                            op=OP.mult)
                        self.cur = st
                        self.off, self.cap = 0, c
                        self.done += c
                    t = self.cur[:, self.off, :]
                    self.off += 1
                    return t

            sb64 = SB(64, C64, eqp64, stp64, iota64, T128, T64)
            sb128 = SB(128, C128, eqp128, stp128, iota128, 0, T128)

            # ---- main loop ----
            ep_bounds = [0, 12, 24, 34, 42, 46, BPC]
            for i in range(BPC):
                acc = psum.tile([128, D], f32, tag="acc")
                nc.tensor.matmul(out=acc[:], lhsT=epsi[:],
                                 rhs=node0_big[:, i, :],
                                 start=True, stop=False,
                                 skip_group_check=True)
                seq = []
                for h in range(2):
                    a, b = int(A[i, h]), int(B[i, h])
                    seq += [(h, 0, 64)] * a + [(h, 0, 128)] + [(h, 64, 64)] * b
                for kt, (h, off, wdt) in enumerate(seq):
                    gs = lo if h == 0 else hi
                    g = next_tile(gs)
                    st = (sb64 if wdt == 64 else sb128).next()
                    if wdt == 128:
                        out_ap = acc[:]
                    else:
                        out_ap = acc[off:off + 64, :]
                    nc.tensor.matmul(out=out_ap, lhsT=st, rhs=g,
                                     start=False, stop=(kt == len(seq) - 1),
                                     skip_group_check=True)
                nc.scalar.activation(out=x_big[:, i, :], in_=acc[:],
                                     func=AF.Copy,
                                     accum_out=sumx[:, i:i + 1])
                nc.scalar.activation(out=sq_scr[:], in_=acc[:],
                                     func=AF.Square,
                                     accum_out=sumsq[:, i:i + 1])

                if i + 1 in ep_bounds:
                    g0 = ep_bounds[ep_bounds.index(i + 1) - 1]
                    g1 = i + 1
                    ng = g1 - g0
                    negmean = epi.tile([128, ng], f32, tag="negmean")
                    nc.scalar.activation(out=negmean[:],
                                         in_=sumx[:, g0:g1], func=AF.Copy,
                                         scale=-1.0 / D)
                    msq = epi.tile([128, ng], f32, tag="msq")
                    nc.vector.tensor_tensor(out=msq[:], in0=negmean[:],
                                            in1=negmean[:], op=OP.mult)
                    var = epi.tile([128, ng], f32, tag="var")
                    nc.scalar.activation(out=var[:], in_=sumsq[:, g0:g1],
                                         func=AF.Copy,
                                         scale=1.0 / D, bias=EPS_LN)
                    nc.vector.tensor_tensor(out=var[:], in0=var[:],
                                            in1=msq[:], op=OP.subtract)
                    std = epi.tile([128, ng], f32, tag="std")
                    nc.scalar.activation(out=std[:], in_=var[:],
                                         func=AF.Sqrt)
                    rstd = epi.tile([128, ng], f32, tag="rstd")
                    nc.vector.reciprocal(rstd[:], std[:])
                    nmr = epi.tile([128, ng], f32, tag="nmr")
                    nc.vector.tensor_tensor(out=nmr[:], in0=negmean[:],
                                            in1=rstd[:], op=OP.mult)
                    xg = x_big[:, g0:g1, :]
                    yg = epi.tile([128, 12, D], f16, tag="y")
                    if gb_identity:
                        for bi in range(ng):
                            nc.scalar.activation(
                                out=yg[:, bi, :],
                                in_=x_big[:, g0 + bi, :],
                                func=AF.Relu,
                                scale=rstd[:, bi:bi + 1],
                                bias=nmr[:, bi:bi + 1])
                    else:
                        nc.vector.tensor_tensor(
                            out=xg, in0=xg,
                            in1=negmean[:].to_broadcast([128, ng, D]),
                            op=OP.add)
                        nc.vector.tensor_tensor(
                            out=xg, in0=xg,
                            in1=rstd[:].to_broadcast([128, ng, D]),
                            op=OP.mult)
                        nc.vector.tensor_tensor(
                            out=xg, in0=xg,
                            in1=gb_bc[:, 0:D].to_broadcast([128, ng, D]),
                            op=OP.mult)
                        nc.vector.tensor_tensor(
                            out=xg, in0=xg,
                            in1=gb_bc[:, D:2 * D].to_broadcast([128, ng, D]),
                            op=OP.add)
                        nc.scalar.activation(out=yg[:, 0:ng, :], in_=xg,
                                             func=AF.Relu)
                    nc.sync.dma_start(
                        out=bass.AP(out_sh, g0 * 128 * D,
                                    [(D, 128), (128 * D, ng), (1, D)]),
                        in_=yg[:, 0:ng, :])
    nc.finalize()
    # Tile assigns DMASW sems round-robin (mod 8) over Pool DMA insts in
    # FINAL scheduled order; a sem must stay locked to one SWDGE queue ->
    # rewrite queue_num to final_order_idx % 4.
    gi = 0
    for bb in nc.m.functions[0].blocks:
        for inst in bb.instructions:
            if type(inst).__name__ == 'InstDMAGatherAnt':
                inst.queue_num = gi % 4
                gi += 1
    return nc


def _pack_gather_idxs(vals, total_tiles):
    ncols = 8 * int(total_tiles)
    arr = np.zeros((16, max(ncols, 8)), np.int16)
    v = np.zeros(int(total_tiles) * 128, np.int16)
    v[:len(vals)] = vals
    col = 0
    done = 0
    while done < total_tiles:
        c = int(min(MAXG, total_tiles - done))
        chunk = v[done * 128:(done + c) * 128]
        arr[:, col:col + 8 * c] = chunk.reshape(8 * c, 16).T
        col += 8 * c
        done += c
    return np.tile(arr, (8, 1))


def _slot_tiles(n0, n1):
    """Static straddle-layout tile counts for one (slot, half) given
    per-core bin counts n0 (dst_local<64), n1 (>=64): [a x W64@0]
    [1 x W128][b x W64@64]."""
    a = max(0, -(-int(n0.max()) // 128) - 1)
    r0 = np.clip(n0 - 128 * a, 0, None)
    b = int((-(-np.clip(r0 + n1 - 128, 0, None) // 128)).max())
    return a, b


def _host_shard(src, dst, w):
    blk = dst >> 7
    local = (dst & 127).astype(np.int64)
    NB = NCORES * BPC
    h_ = (src >= HALF).astype(np.int64)
    b_ = (local >= 64).astype(np.int64)
    cnt4 = np.zeros((NB, 2, 2), np.int64)
    np.add.at(cnt4, (blk, h_, b_), 1)
    block2core, block2slot = _assign_blocks(cnt4)
    key = (((block2core[blk] * BPC + block2slot[blk]) * 2 + h_) * 2 + b_)
    order = np.lexsort((src, key))
    src_s = src[order].astype(np.int32)
    dst_s = dst[order].astype(np.int32)
    local_s = (dst_s & 127).astype(np.int32)
    w_s = w[order]
    cnt = np.bincount(key[order], minlength=4 * NB)
    offs = np.concatenate([[0], np.cumsum(cnt)])
    # per (core, slot, half, bin) counts
    C = np.zeros((NCORES, BPC, 2, 2), np.int64)
    for bb in range(NB):
        C[block2core[bb], block2slot[bb]] = cnt4[bb]
    A = np.zeros((BPC, 2), np.int64)
    B = np.zeros((BPC, 2), np.int64)
    for s in range(BPC):
        for h in range(2):
            A[s, h], B[s, h] = _slot_tiles(C[:, s, h, 0], C[:, s, h, 1])
    # big slots first: the pipeline tail then drains on the smallest slots
    slot_order = np.argsort(-(A.sum(1) + B.sum(1)), kind="stable")
    inv = np.empty(BPC, np.int64)
    inv[slot_order] = np.arange(BPC)
    block2slot = inv[block2slot]
    A = A[slot_order].copy()
    B = B[slot_order].copy()
    # offs/key were built with OLD slot ids; rebuild ordering arrays
    return (block2core, block2slot, offs, src_s, dst_s, local_s, w_s, A, B,
            slot_order)


def _assign_blocks(cnt4):
    """Partition the 392 dst blocks into 49 slot-groups of 8 (one per core;
    which core is arbitrary since the host reorders output blocks).
    Objective: total straddle-layout tiles. Initial: size-sorted strips of
    8; then local search with cross-group member swaps."""
    NB = NCORES * BPC
    tot = cnt4.sum(axis=(1, 2))
    order = np.argsort(-tot, kind="stable")
    groups = order.reshape(BPC, NCORES).copy()

    c4 = [tuple(int(x) for x in cnt4[b].reshape(4)) for b in range(NB)]
    grp = [list(groups[g]) for g in range(BPC)]

    def gcost(members):
        t = 0
        for h in range(2):
            mx0 = 0
            for m in members:
                v = c4[m][2 * h]
                if v > mx0:
                    mx0 = v
            a = -(-mx0 // 128) - 1
            if a < 0:
                a = 0
            mxb = 0
            for m in members:
                c = c4[m]
                r0 = c[2 * h] - 128 * a
                if r0 < 0:
                    r0 = 0
                rem = r0 + c[2 * h + 1] - 128
                if rem < 0:
                    rem = 0
                bb = -(-rem // 128)
                if bb > mxb:
                    mxb = bb
            t += a + 1 + mxb
        return t

    cost = [gcost(grp[g]) for g in range(BPC)]
    rng = np.random.default_rng(11)
    NIT = 300000
    ga_ = rng.integers(0, BPC, NIT)
    gb_ = rng.integers(0, BPC, NIT)
    ia_ = rng.integers(0, NCORES, NIT)
    ib_ = rng.integers(0, NCORES, NIT)
    for ga, gb, ia, ib in zip(ga_, gb_, ia_, ib_):
        if ga == gb:
            continue
        old = cost[ga] + cost[gb]
        grp[ga][ia], grp[gb][ib] = grp[gb][ib], grp[ga][ia]
        ca, cb = gcost(grp[ga]), gcost(grp[gb])
        if ca + cb <= old:
            cost[ga], cost[gb] = ca, cb
        else:
            grp[ga][ia], grp[gb][ib] = grp[gb][ib], grp[ga][ia]
    block2core = np.empty(NB, np.int64)
    block2slot = np.empty(NB, np.int64)
    for g in range(BPC):
        for m in range(NCORES):
            block2core[grp[g][m]] = m
            block2slot[grp[g][m]] = g
    return block2core, block2slot


def _pads(m):
    """m scattered ascending int16 idxs for gather pad slots."""
    return np.sort((np.arange(m, dtype=np.int64) * 97 % HALF)).astype(np.int16)


def _build_in_b(node_0, aug_full, al_full, ar_full,
                block2core, block2slot, offs, src_s, dst_s, local_s, w_s,
                A, B, slot_order, ln_weight, ln_bias):
    NB = NCORES * BPC
    TL, TH, T64, T128 = _tile_counts(A, B)
    TT = T64 + T128
    node0_pad = np.zeros((NPAD, D), np.float32)
    node0_pad[:N] = node_0
    gbv = np.concatenate([ln_weight, ln_bias])[None, :]
    iota64_np = np.tile(np.arange(64, dtype=np.float16)[None, :], (128, 1))
    iota128_np = np.tile(np.arange(128, dtype=np.float16)[None, :], (128, 1))
    epsi_np = (EPS_FA * np.eye(128, dtype=np.float32)).astype(np.float16)
    al16 = al_full.astype(np.float16)
    ar16 = ar_full.astype(np.float16)
    w16 = w_s.astype(np.float16)
    in_b = []
    for k in range(NCORES):
        lo_vals, hi_vals = [], []
        dstl_arr = np.zeros((128, TT), np.float16)
        w_arr = np.zeros((128, TT), np.float16)
        al_arr = np.zeros((128, TT), np.float16)
        ar_arr = np.zeros((128, TT), np.float16)
        col64 = T128
        col128 = 0

        def put_cols(col, ntile, sl_list, sent, dsub, h):
            """Fill ntile columns starting at col from edge slices sl_list
            (concatenated, in order). dsub subtracted from dst_local;
            sent = pad sentinel; h picks the gather base half."""
            idxs = np.concatenate([np.arange(s.start, s.stop)
                                   for s in sl_list]) if sl_list else \
                np.zeros(0, np.int64)
            nv = len(idxs)
            cap = ntile * 128
            dv = np.full(cap, sent, np.float16)
            dv[:nv] = local_s[idxs] - dsub
            wv = np.zeros(cap, np.float16)
            wv[:nv] = w16[idxs]
            av = np.zeros(cap, np.float16)
            av[:nv] = al16[src_s[idxs]]
            rv = np.zeros(cap, np.float16)
            rv[:nv] = ar16[dst_s[idxs]]
            dstl_arr[:, col:col + ntile] = dv.reshape(ntile, 128).T
            w_arr[:, col:col + ntile] = wv.reshape(ntile, 128).T
            al_arr[:, col:col + ntile] = av.reshape(ntile, 128).T
            ar_arr[:, col:col + ntile] = rv.reshape(ntile, 128).T
            iv = _pads(cap)
            iv[:nv] = (src_s[idxs] - HALF * h).astype(np.int16)
            return iv

        for i in range(BPC):
            for h in range(2):
                ki = ((k * BPC + int(slot_order[i])) * 2 + h) * 2
                s0, s1, s2 = offs[ki], offs[ki + 1], offs[ki + 2]
                n0, n1 = s1 - s0, s2 - s1
                a, b = int(A[i, h]), int(B[i, h])
                m0 = min(128 * a, n0)
                r0 = n0 - m0
                stk = min(128 - r0, n1)
                coll = lo_vals if h == 0 else hi_vals
                # W64 @0 tiles (bin0 head)
                iv = put_cols(col64, a, [slice(s0, s0 + m0)] if a else [],
                              100.0, 0, h)
                if a:
                    coll.append(iv)
                # straddle (bin0 tail + bin1 head)
                iv = put_cols(col128, 1,
                              [slice(s0 + m0, s1), slice(s1, s1 + stk)],
                              200.0, 0, h)
                coll.append(iv)
                # W64 @64 tiles (bin1 tail)
                iv = put_cols(col64 + a, b,
                              [slice(s1 + stk, s2)] if b else [], 100.0, 64,
                              h)
                if b:
                    coll.append(iv)
                col64 += a + b
                col128 += 1
        blocks_k = np.array([np.where((block2core == k) & (block2slot == i))[0][0]
                             for i in range(BPC)])
        node0_k = node0_pad.reshape(NB, 128, D)[blocks_k].reshape(NSH, D)
        in_b.append({
            "aug": aug_full,
            "idx_lo": _pack_gather_idxs(np.concatenate(lo_vals), TL),
            "idx_hi": _pack_gather_idxs(np.concatenate(hi_vals), TH),
            "dstl": dstl_arr,
            "wgt": w_arr,
            "alv": al_arr,
            "arv": ar_arr,
            "node0_sh": node0_k.astype(np.float16),
            "gb": gbv,
            "iota64_in": iota64_np,
            "iota128_in": iota128_np,
            "epsi_in": epsi_np,
        })
        _cache.setdefault("blocks_by_core", {})[k] = blocks_k
    return in_b


def kernel(node, node_0, edge_index, edge_attr, batch_ptr,
           att_l, att_r, ln_weight, ln_bias):
    node = np.asarray(node, np.float32)
    node_0 = np.asarray(node_0, np.float32)
    src = np.asarray(edge_index[0], np.int64)
    dst = np.asarray(edge_index[1], np.int64)
    w = np.asarray(edge_attr, np.float32)
    att_l = np.asarray(att_l, np.float32)
    att_r = np.asarray(att_r, np.float32)
    ln_weight = np.asarray(ln_weight, np.float32)
    ln_bias = np.asarray(ln_bias, np.float32)

    (block2core, block2slot, offs, src_s, dst_s, local_s, w_s,
     A, B, slot_order) = _host_shard(src, dst, w)

    gb_identity = bool(np.all(ln_weight == 1.0) and np.all(ln_bias == 0.0))
    sig = (A.tobytes(), B.tobytes(), gb_identity)
    if "A" not in _cache:
        _cache["A"] = _build_phase_a()
    if ("B", sig) not in _cache:
        _cache[("B", sig)] = _build_phase_b(A, B, gb_identity)
    nc_a = _cache["A"]
    nc_b = _cache[("B", sig)]

    # ---- phase A ----
    node_pad = np.zeros((NPAD, D), np.float32)
    node_pad[:N] = node
    att = np.stack([att_l, att_r])
    in_a = [{"node_sh": node_pad[k * NSH:(k + 1) * NSH], "att": att}
            for k in range(NCORES)]
    res_a = run_bass_kernel_spmd(nc_a, in_a, list(range(NCORES)),
                                 **_cache.get("runkw", {}))
    aug_full = np.concatenate([res_a.results[k]["aug_sh"]
                               for k in range(NCORES)])
    al_full = np.concatenate(
        [res_a.results[k]["alr_sh"][:, 0:BPC].T.reshape(NSH)
         for k in range(NCORES)])
    ar_full = np.concatenate(
        [res_a.results[k]["alr_sh"][:, BPC:2 * BPC].T.reshape(NSH)
         for k in range(NCORES)])
    t_a = res_a.exec_time_ns

    # ---- phase B ----
    in_b = _build_in_b(node_0, aug_full, al_full, ar_full,
                       block2core, block2slot, offs, src_s, dst_s, local_s,
                       w_s, A, B, slot_order, ln_weight, ln_bias)
    res_b = run_bass_kernel_spmd(nc_b, in_b, list(range(NCORES)),
                                 **_cache.get("runkw", {}))
    NB = NCORES * BPC
    out = np.empty((NB, 128, D), np.float32)
    for k in range(NCORES):
        out[_cache["blocks_by_core"][k]] = \
            res_b.results[k]["out_sh"].astype(np.float32).reshape(BPC, 128, D)
    out = out.reshape(NPAD, D)
    t_b = res_b.exec_time_ns
    _cache["t_a_ns"] = t_a
    _cache["t_b_ns"] = t_b
    _cache["res_a"] = res_a
    _cache["res_b"] = res_b
    if t_a is not None and t_b is not None:
        _cache["last_exec_ns"] = t_a + t_b
    return out[:N]


# revision 24
# speedup vs baseline: 1.0173x; 1.0107x over previous
"""FAConv + LayerNorm + ReLU fused Trainium2 kernel (8 NeuronCores, SPMD).

Strategy (v3):
  Host: sort edges by destination 128-node block (core k owns 49 blocks =
  a contiguous 6272-node output shard -> no all-reduce), split each block's
  edges by src < 25088 (int16 gather-index limit) and by dst_local < 64
  (one-hot window), pack per (block, half) as [a x W64@0][1 x W128]
  [b x W64@64] tiles of 128 edges (straddle tile absorbs bin remainders ->
  ~zero SPMD padding), edges sorted by src within regions for HBM gather
  locality.
  Phase A (data-parallel): whole-shard SBUF residency; a_l/a_r = node @ att
  via broadcast DVE multiply + reductions; bf16 node table cast on Scalar;
  3 big DMAs total.
  Host: concat shards; permute a_l by edge src and a_r by edge dst into the
  padded tile layout as float16 streams (data movement only).
  Phase B (edge-parallel): coef = tanh(a_l+a_r)*w whole-array; one-hot stat
  tiles built 64-wide (or 128-wide for straddles) with stride-0 broadcast
  APs; src rows dma_gathered in 1024-row calls; segment-sum as
  PSUM-accumulated matmuls writing 64-partition windows (0.1*I identity
  matmul folds the eps*node_0 skip and zeroes PSUM); LayerNorm stats
  accumulated per block by 2 Scalar ops during PSUM drain; normalization +
  ReLU whole-array at the end.
"""
import sys

for _p in ('/opt/trn_rl_repo', '/root/.axon_site/_ro/trn_rl_repo'):
    if _p not in sys.path:
        sys.path.insert(0, _p)

import numpy as np
import ml_dtypes

import concourse.bass as bass
import concourse.bacc as bacc
import concourse.tile as tile
from concourse import mybir
from concourse.bass_utils import run_bass_kernel_spmd

N = 50000
D = 256
NCORES = 8
BPC = 49                    # dst blocks per core
NPAD = NCORES * BPC * 128   # 50176
NSH = BPC * 128             # 6272 nodes per core shard
HALF = NPAD // 2            # 25088 (int16-safe gather index range)
EPS_FA = 0.1
EPS_LN = 1e-5
MAXG = 8                    # tiles (of 128 idxs) per dma_gather call
C64 = 32                    # W64 tiles per batched one-hot build
C128 = 16                   # straddle tiles per batched build

f32 = mybir.dt.float32
bf16 = mybir.dt.bfloat16
f16 = mybir.dt.float16
i16 = mybir.dt.int16
AF = mybir.ActivationFunctionType
OP = mybir.AluOpType

_cache = {}


def _shard_ap(t):
    return bass.AP(t, 0, [(D, 128), (128 * D, BPC), (1, D)])


def _build_phase_a():
    nc = bacc.Bacc("TRN2", target_bir_lowering=False, debug=False,
                   num_devices=NCORES)
    node_sh = nc.declare_dram_parameter("node_sh", [NSH, D], f32, isOutput=False)
    att = nc.declare_dram_parameter("att", [2, D], f32, isOutput=False)
    aug_sh = nc.declare_dram_parameter("aug_sh", [NSH, D], f16, isOutput=True)
    alr_sh = nc.declare_dram_parameter("alr_sh", [128, 2 * BPC], f16, isOutput=True)

    with tile.TileContext(nc) as tc:
        with (
            tc.tile_pool(name="const", bufs=1) as cpool,
            tc.tile_pool(name="big", bufs=1) as big,
            tc.tile_pool(name="psum", bufs=2, space="PSUM") as psum,
        ):
            ones = cpool.tile([1, 128], f32)
            nc.vector.memset(ones[:], 1.0)
            att_bc = []
            for j in range(2):
                att_row = cpool.tile([1, D], f32, tag=f"attrow{j}")
                nc.sync.dma_start(out=att_row[:], in_=att[j:j + 1, :])
                ps = psum.tile([128, D], f32, tag="attps")
                nc.tensor.matmul(out=ps[:], lhsT=ones[:], rhs=att_row[:],
                                 start=True, stop=True)
                bc = cpool.tile([128, 1, D], f16, tag=f"attbc{j}")
                nc.vector.tensor_copy(bc[:, 0, :], ps[:])
                att_bc.append(bc)

            NCHUNK = 8
            bounds = [round(BPC * i / NCHUNK) for i in range(NCHUNK + 1)]
            alr = big.tile([128, 2 * BPC], f16, tag="alr")
            for ci in range(NCHUNK):
                b0, b1 = bounds[ci], bounds[ci + 1]
                nb = b1 - b0
                node_c = big.tile([128, nb, D], f32, tag=f"node{ci}")
                nc.sync.dma_start(
                    out=node_c[:],
                    in_=bass.AP(node_sh, b0 * 128 * D,
                                [(D, 128), (128 * D, nb), (1, D)]))
                aug_c = big.tile([128, nb, D], f16, tag=f"aug{ci}")
                nc.scalar.activation(out=aug_c[:], in_=node_c[:], func=AF.Copy)
                nc.sync.dma_start(
                    out=bass.AP(aug_sh, b0 * 128 * D,
                                [(D, 128), (128 * D, nb), (1, D)]),
                    in_=aug_c[:])
                # a_l: DVE mult + X-reduce.  a_r: mult on Pool (its only
                # job); reduce split Scalar-accum / DVE by measured rates.
                scr_l = big.tile([128, nb, D], f16, tag=f"scrl{ci}")
                nc.vector.tensor_tensor(
                    out=scr_l[:], in0=aug_c[:],
                    in1=att_bc[0][:].to_broadcast([128, nb, D]),
                    op=OP.mult)
                scr_r = big.tile([128, nb, D], f16, tag=f"scrr{ci}")
                nc.gpsimd.tensor_tensor(
                    out=scr_r[:], in0=aug_c[:],
                    in1=att_bc[1][:].to_broadcast([128, nb, D]),
                    op=OP.mult)
                with nc.allow_low_precision(
                        reason="f16 store of O(1) dot products; "
                               "reduce accumulates in f32"):
                    nc.vector.tensor_reduce(
                        out=alr[:, b0:b1], in_=scr_l[:],
                        axis=mybir.AxisListType.X, op=OP.add)
                    if ci >= 5:
                        nc.vector.tensor_reduce(
                            out=alr[:, BPC + b0:BPC + b1], in_=scr_r[:],
                            axis=mybir.AxisListType.X, op=OP.add)
                if ci < 5:
                    with nc.allow_low_precision(
                            reason="f16 store of O(1) dot products; "
                                   "Act accumulator is f32"):
                        for bi in range(nb):
                            nc.scalar.activation(
                                out=scr_r[:, bi, :], in_=scr_r[:, bi, :],
                                func=AF.Copy,
                                accum_out=alr[:, BPC + b0 + bi:
                                              BPC + b0 + bi + 1])
            nc.sync.dma_start(out=alr_sh[:, :], in_=alr[:])
    nc.finalize()
    return nc


def _tile_counts(A, B):
    TL = int((A[:, 0] + B[:, 0]).sum()) + BPC
    TH = int((A[:, 1] + B[:, 1]).sum()) + BPC
    T64 = int((A + B).sum())
    T128 = 2 * BPC
    return TL, TH, T64, T128


def _build_phase_b(A, B, gb_identity):
    TL, TH, T64, T128 = _tile_counts(A, B)
    TT = T64 + T128
    nc = bacc.Bacc("TRN2", target_bir_lowering=False, debug=False,
                   num_devices=NCORES, num_swdge_queues=4)
    aug = nc.declare_dram_parameter("aug", [NPAD, D], f16, isOutput=False)
    idx_lo = nc.declare_dram_parameter("idx_lo", [128, max(8 * TL, 8)], i16,
                                       isOutput=False)
    idx_hi = nc.declare_dram_parameter("idx_hi", [128, max(8 * TH, 8)], i16,
                                       isOutput=False)
    dstl = nc.declare_dram_parameter("dstl", [128, TT], f16, isOutput=False)
    wgt = nc.declare_dram_parameter("wgt", [128, TT], f16, isOutput=False)
    alv = nc.declare_dram_parameter("alv", [128, TT], f16, isOutput=False)
    arv = nc.declare_dram_parameter("arv", [128, TT], f16, isOutput=False)
    node0_sh = nc.declare_dram_parameter("node0_sh", [NSH, D], f16,
                                         isOutput=False)
    gb = nc.declare_dram_parameter("gb", [1, 2 * D], f32, isOutput=False)
    iota64_in = nc.declare_dram_parameter("iota64_in", [128, 64], f16,
                                          isOutput=False)
    iota128_in = nc.declare_dram_parameter("iota128_in", [128, 128], f16,
                                           isOutput=False)
    epsi_in = nc.declare_dram_parameter("epsi_in", [128, 128], f16,
                                        isOutput=False)
    out_sh = nc.declare_dram_parameter("out_sh", [NSH, D], f16, isOutput=True)

    with tile.TileContext(nc) as tc:
        with (
            tc.tile_pool(name="const", bufs=1) as cpool,
            tc.tile_pool(name="big", bufs=1) as big,
            tc.tile_pool(name="glo", bufs=6) as glo,
            tc.tile_pool(name="ghi", bufs=6) as ghi,
            tc.tile_pool(name="eq64", bufs=2) as eqp64,
            tc.tile_pool(name="st64", bufs=2) as stp64,
            tc.tile_pool(name="eq128", bufs=2) as eqp128,
            tc.tile_pool(name="st128", bufs=2) as stp128,
            tc.tile_pool(name="epi", bufs=2) as epi,
            tc.tile_pool(name="psum", bufs=4, space="PSUM") as psum,
            tc.tile_pool(name="gbps", bufs=1, space="PSUM") as gbpsum,
        ):
            # ---- gather idx streams first: nothing else gates the gathers.
            # Small heads via the (idle) Scalar engine DMA queue so the
            # first gather calls start ~1us after boot; bulk via Sync.
            IHEAD = 8 * 4 * MAXG          # first ~4 calls' columns
            ilo = cpool.tile([128, max(8 * TL, 8)], i16, tag="ilo")
            nlo = max(8 * TL, 8)
            nc.scalar.dma_start(out=ilo[:, 0:min(IHEAD, nlo)],
                                in_=idx_lo[:, 0:min(IHEAD, nlo)])
            if nlo > IHEAD:
                nc.sync.dma_start(out=ilo[:, IHEAD:nlo],
                                  in_=idx_lo[:, IHEAD:nlo])
            ihi = cpool.tile([128, max(8 * TH, 8)], i16, tag="ihi")
            nhi = max(8 * TH, 8)
            nc.scalar.dma_start(out=ihi[:, 0:min(IHEAD, nhi)],
                                in_=idx_hi[:, 0:min(IHEAD, nhi)])
            if nhi > IHEAD:
                nc.sync.dma_start(out=ihi[:, IHEAD:nhi],
                                  in_=idx_hi[:, IHEAD:nhi])

            # ---- constants ----
            iota64 = cpool.tile([128, 1, 64], f16)
            nc.sync.dma_start(out=iota64[:, 0, :], in_=iota64_in[:, :])
            iota128 = cpool.tile([128, 1, 128], f16)
            nc.sync.dma_start(out=iota128[:, 0, :], in_=iota128_in[:, :])
            epsi = cpool.tile([128, 128], f16)
            nc.sync.dma_start(out=epsi[:], in_=epsi_in[:, :])
            if not gb_identity:
                ones_f = cpool.tile([1, 128], f32)
                nc.vector.memset(ones_f[:], 1.0)
                gb_row = cpool.tile([1, 2 * D], f32)
                nc.sync.dma_start(out=gb_row[:], in_=gb[:, :])
                gb_ps = gbpsum.tile([128, 2 * D], f32, tag="gbps")
                nc.tensor.matmul(out=gb_ps[:], lhsT=ones_f[:], rhs=gb_row[:],
                                 start=True, stop=True)
                gb_bc = cpool.tile([128, 2 * D], f32)
                nc.vector.tensor_copy(gb_bc[:], gb_ps[:])

            # ---- stream preload: priority head so the first stat chunks
            # and coef columns are ready early, remainder via Sync ----
            PRI = T128 + 2 * C64
            dstl_sb = cpool.tile([128, TT], f16, tag="dstl")
            nc.scalar.dma_start(out=dstl_sb[:, 0:PRI], in_=dstl[:, 0:PRI])
            nc.sync.dma_start(out=dstl_sb[:, PRI:TT], in_=dstl[:, PRI:TT])
            w_sb = cpool.tile([128, TT], f16, tag="w")
            nc.scalar.dma_start(out=w_sb[:, 0:PRI], in_=wgt[:, 0:PRI])
            nc.sync.dma_start(out=w_sb[:, PRI:TT], in_=wgt[:, PRI:TT])
            al_sb = cpool.tile([128, TT], f16, tag="al")
            nc.scalar.dma_start(out=al_sb[:, 0:PRI], in_=alv[:, 0:PRI])
            nc.sync.dma_start(out=al_sb[:, PRI:TT], in_=alv[:, PRI:TT])
            ar_sb = cpool.tile([128, TT], f16, tag="ar")
            nc.scalar.dma_start(out=ar_sb[:, 0:PRI], in_=arv[:, 0:PRI])
            nc.sync.dma_start(out=ar_sb[:, PRI:TT], in_=arv[:, PRI:TT])
            node0_big = big.tile([128, BPC, D], f16, tag="node0")
            n0_bounds = [0, 4, 16, BPC]
            for b0, b1 in zip(n0_bounds[:-1], n0_bounds[1:]):
                nc.sync.dma_start(
                    out=node0_big[:, b0:b1, :],
                    in_=bass.AP(node0_sh, b0 * 128 * D,
                                [(D, 128), (128 * D, b1 - b0), (1, D)]))

            # ---- whole-array coef = tanh(al + ar) * w, priority head first
            arg_sb = cpool.tile([128, TT], f32, tag="arg")
            tanh16 = cpool.tile([128, TT], f16, tag="tanh16")
            coef_sb = cpool.tile([128, TT], f16, tag="coef")
            for c0, c1 in ((0, PRI), (PRI, TT)):
                nc.vector.tensor_tensor(out=arg_sb[:, c0:c1],
                                        in0=al_sb[:, c0:c1],
                                        in1=ar_sb[:, c0:c1], op=OP.add)
                nc.scalar.activation(out=tanh16[:, c0:c1],
                                     in_=arg_sb[:, c0:c1], func=AF.Tanh)
                nc.vector.tensor_tensor(out=coef_sb[:, c0:c1],
                                        in0=tanh16[:, c0:c1],
                                        in1=w_sb[:, c0:c1], op=OP.mult)

            # ---- LN stat accumulators + x staging ----
            x_big = big.tile([128, BPC, D], f32, tag="x")
            sumx = big.tile([128, BPC], f32, tag="sumx")
            sumsq = big.tile([128, BPC], f32, tag="sumsq")
            sq_scr = epi.tile([128, D], f32, tag="sqscr")

            # ---- gather stream state (as v2) ----
            qctr = [0]

            class GS:
                def __init__(self, pool, isb, total):
                    self.pool, self.isb, self.total = pool, isb, total
                    self.col = 0
                    self.done = 0
                    self.gbt = None
                    self.slot = 0
                    self.cap = 0

            def next_tile(gs):
                if gs.gbt is None or gs.slot == gs.cap:
                    c = min(MAXG, gs.total - gs.done)
                    gs.gbt = gs.pool.tile([128, MAXG, D], f16, tag="g")
                    nc.gpsimd.dma_gather(
                        out_ap=gs.gbt[:, 0:c, :], in_ap=gs.base,
                        idxs_ap=gs.isb[:, gs.col:gs.col + 8 * c],
                        num_idxs=c * 128, num_idxs_reg=c * 128,
                        elem_size=D,
                        queue_num=qctr[0] % 4)
                    qctr[0] += 1
                    gs.col += 8 * c
                    gs.done += c
                    gs.slot, gs.cap = 0, c
                t = gs.gbt[:, gs.slot, :]
                gs.slot += 1
                return t

            lo = GS(glo, ilo, TL)
            lo.base = aug[0:HALF, :]
            hi = GS(ghi, ihi, TH)
            hi.base = aug[HALF:NPAD, :]

            # ---- one-hot stat builders (batched) ----
            class SB:
                def __init__(self, width, csz, eqp, stp, iota_bc, base, total):
                    self.width, self.csz = width, csz
                    self.eqp, self.stp, self.iota_bc = eqp, stp, iota_bc
                    self.base, self.total = base, total
                    self.done = 0
                    self.off = 0
                    self.cap = 0
                    self.cur = None

                def next(self):
                    if self.cur is None or self.off == self.cap:
                        c = min(self.csz, self.total - self.done)
                        col = self.base + self.done
                        eq = self.eqp.tile([128, self.csz, self.width], f16,
                                           tag="eq")
                        nc.vector.tensor_tensor(
                            out=eq[:, 0:c, :],
                            in0=self.iota_bc[:].to_broadcast(
                                [128, c, self.width]),
                            in1=dstl_sb[:, col:col + c].to_broadcast(
                                [128, c, self.width]),
                            op=OP.is_equal)
                        st = self.stp.tile([128, self.csz, self.width], f16,
                                           tag="st")
                        nc.vector.tensor_tensor(
                            out=st[:, 0:c, :],
                            in0=eq[:, 0:c, :],
                            in1=coef_sb[:, col:col + c].to_broadcast(
                                [128, c, self.width]),
                            op=OP.mult)
                        self.cur = st
                        self.off, self.cap = 0, c
                        self.done += c
                    t = self.cur[:, self.off, :]
                    self.off += 1
                    return t

            sb64 = SB(64, C64, eqp64, stp64, iota64, T128, T64)
            sb128 = SB(128, C128, eqp128, stp128, iota128, 0, T128)

            # ---- main loop ----
            ep_bounds = [0, 12, 24, 34, 42, 46, 48, BPC]
            for i in range(BPC):
                acc = psum.tile([128, D], f32, tag="acc")
                nc.tensor.matmul(out=acc[:], lhsT=epsi[:],
                                 rhs=node0_big[:, i, :],
                                 start=True, stop=False,
                                 skip_group_check=True)
                seq = []
                for h in range(2):
                    a, b = int(A[i, h]), int(B[i, h])
                    seq += [(h, 0, 64)] * a + [(h, 0, 128)] + [(h, 64, 64)] * b
                for kt, (h, off, wdt) in enumerate(seq):
                    gs = lo if h == 0 else hi
                    g = next_tile(gs)
                    st = (sb64 if wdt == 64 else sb128).next()
                    if wdt == 128:
                        out_ap = acc[:]
                    else:
                        out_ap = acc[off:off + 64, :]
                    nc.tensor.matmul(out=out_ap, lhsT=st, rhs=g,
                                     start=False, stop=(kt == len(seq) - 1),
                                     skip_group_check=True)
                nc.scalar.activation(out=x_big[:, i, :], in_=acc[:],
                                     func=AF.Copy,
                                     accum_out=sumx[:, i:i + 1])
                nc.scalar.activation(out=sq_scr[:], in_=acc[:],
                                     func=AF.Square,
                                     accum_out=sumsq[:, i:i + 1])

                if i + 1 in ep_bounds:
                    g0 = ep_bounds[ep_bounds.index(i + 1) - 1]
                    g1 = i + 1
                    ng = g1 - g0
                    negmean = epi.tile([128, ng], f32, tag="negmean")
                    nc.scalar.activation(out=negmean[:],
                                         in_=sumx[:, g0:g1], func=AF.Copy,
                                         scale=-1.0 / D)
                    msq = epi.tile([128, ng], f32, tag="msq")
                    nc.vector.tensor_tensor(out=msq[:], in0=negmean[:],
                                            in1=negmean[:], op=OP.mult)
                    var = epi.tile([128, ng], f32, tag="var")
                    nc.scalar.activation(out=var[:], in_=sumsq[:, g0:g1],
                                         func=AF.Copy,
                                         scale=1.0 / D, bias=EPS_LN)
                    nc.vector.tensor_tensor(out=var[:], in0=var[:],
                                            in1=msq[:], op=OP.subtract)
                    std = epi.tile([128, ng], f32, tag="std")
                    nc.scalar.activation(out=std[:], in_=var[:],
                                         func=AF.Sqrt)
                    rstd = epi.tile([128, ng], f32, tag="rstd")
                    nc.vector.reciprocal(rstd[:], std[:])
                    nmr = epi.tile([128, ng], f32, tag="nmr")
                    nc.vector.tensor_tensor(out=nmr[:], in0=negmean[:],
                                            in1=rstd[:], op=OP.mult)
                    xg = x_big[:, g0:g1, :]
                    yg = epi.tile([128, 12, D], f16, tag="y")
                    if gb_identity:
                        for bi in range(ng):
                            nc.scalar.activation(
                                out=yg[:, bi, :],
                                in_=x_big[:, g0 + bi, :],
                                func=AF.Relu,
                                scale=rstd[:, bi:bi + 1],
                                bias=nmr[:, bi:bi + 1])
                    else:
                        nc.vector.tensor_tensor(
                            out=xg, in0=xg,
                            in1=negmean[:].to_broadcast([128, ng, D]),
                            op=OP.add)
                        nc.vector.tensor_tensor(
                            out=xg, in0=xg,
                            in1=rstd[:].to_broadcast([128, ng, D]),
                            op=OP.mult)
                        nc.vector.tensor_tensor(
                            out=xg, in0=xg,
                            in1=gb_bc[:, 0:D].to_broadcast([128, ng, D]),
                            op=OP.mult)
                        nc.vector.tensor_tensor(
                            out=xg, in0=xg,
                            in1=gb_bc[:, D:2 * D].to_broadcast([128, ng, D]),
                            op=OP.add)
                        nc.scalar.activation(out=yg[:, 0:ng, :], in_=xg,
                                             func=AF.Relu)
                    nc.sync.dma_start(
                        out=bass.AP(out_sh, g0 * 128 * D,
                                    [(D, 128), (128 * D, ng), (1, D)]),
                        in_=yg[:, 0:ng, :])
    nc.finalize()
    # Tile assigns DMASW sems round-robin (mod 8) over Pool DMA insts in
    # FINAL scheduled order; a sem must stay locked to one SWDGE queue ->
    # rewrite queue_num to final_order_idx % 4.
    gi = 0
    for bb in nc.m.functions[0].blocks:
        for inst in bb.instructions:
            if type(inst).__name__ == 'InstDMAGatherAnt':
                inst.queue_num = gi % 4
                gi += 1
    return nc


def _pack_gather_idxs(vals, total_tiles):
    ncols = 8 * int(total_tiles)
    arr = np.zeros((16, max(ncols, 8)), np.int16)
    v = np.zeros(int(total_tiles) * 128, np.int16)
    v[:len(vals)] = vals
    col = 0
    done = 0
    while done < total_tiles:
        c = int(min(MAXG, total_tiles - done))
        chunk = v[done * 128:(done + c) * 128]
        arr[:, col:col + 8 * c] = chunk.reshape(8 * c, 16).T
        col += 8 * c
        done += c
    return np.tile(arr, (8, 1))


def _slot_tiles(n0, n1):
    """Static straddle-layout tile counts for one (slot, half) given
    per-core bin counts n0 (dst_local<64), n1 (>=64): [a x W64@0]
    [1 x W128][b x W64@64]."""
    a = max(0, -(-int(n0.max()) // 128) - 1)
    r0 = np.clip(n0 - 128 * a, 0, None)
    b = int((-(-np.clip(r0 + n1 - 128, 0, None) // 128)).max())
    return a, b


def _host_shard(src, dst, w):
    blk = dst >> 7
    local = (dst & 127).astype(np.int64)
    NB = NCORES * BPC
    h_ = (src >= HALF).astype(np.int64)
    b_ = (local >= 64).astype(np.int64)
    cnt4 = np.zeros((NB, 2, 2), np.int64)
    np.add.at(cnt4, (blk, h_, b_), 1)
    block2core, block2slot = _assign_blocks(cnt4)
    key = (((block2core[blk] * BPC + block2slot[blk]) * 2 + h_) * 2 + b_)
    order = np.lexsort((src, key))
    src_s = src[order].astype(np.int32)
    dst_s = dst[order].astype(np.int32)
    local_s = (dst_s & 127).astype(np.int32)
    w_s = w[order]
    cnt = np.bincount(key[order], minlength=4 * NB)
    offs = np.concatenate([[0], np.cumsum(cnt)])
    # per (core, slot, half, bin) counts
    C = np.zeros((NCORES, BPC, 2, 2), np.int64)
    for bb in range(NB):
        C[block2core[bb], block2slot[bb]] = cnt4[bb]
    A = np.zeros((BPC, 2), np.int64)
    B = np.zeros((BPC, 2), np.int64)
    for s in range(BPC):
        for h in range(2):
            A[s, h], B[s, h] = _slot_tiles(C[:, s, h, 0], C[:, s, h, 1])
    # big slots first: the pipeline tail then drains on the smallest slots
    slot_order = np.argsort(-(A.sum(1) + B.sum(1)), kind="stable")
    inv = np.empty(BPC, np.int64)
    inv[slot_order] = np.arange(BPC)
    block2slot = inv[block2slot]
    A = A[slot_order].copy()
    B = B[slot_order].copy()
    # offs/key were built with OLD slot ids; rebuild ordering arrays
    return (block2core, block2slot, offs, src_s, dst_s, local_s, w_s, A, B,
            slot_order)


def _assign_blocks(cnt4):
    """Partition the 392 dst blocks into 49 slot-groups of 8 (one per core;
    which core is arbitrary since the host reorders output blocks).
    Objective: total straddle-layout tiles. Initial: size-sorted strips of
    8; then local search with cross-group member swaps."""
    NB = NCORES * BPC
    tot = cnt4.sum(axis=(1, 2))
    order = np.argsort(-tot, kind="stable")
    groups = order.reshape(BPC, NCORES).copy()

    c4 = [tuple(int(x) for x in cnt4[b].reshape(4)) for b in range(NB)]
    grp = [list(groups[g]) for g in range(BPC)]

    def gcost(members):
        t = 0
        for h in range(2):
            mx0 = 0
            for m in members:
                v = c4[m][2 * h]
                if v > mx0:
                    mx0 = v
            a = -(-mx0 // 128) - 1
            if a < 0:
                a = 0
            mxb = 0
            for m in members:
                c = c4[m]
                r0 = c[2 * h] - 128 * a
                if r0 < 0:
                    r0 = 0
                rem = r0 + c[2 * h + 1] - 128
                if rem < 0:
                    rem = 0
                bb = -(-rem // 128)
                if bb > mxb:
                    mxb = bb
            t += a + 1 + mxb
        return t

    cost = [gcost(grp[g]) for g in range(BPC)]
    rng = np.random.default_rng(11)
    NIT = 300000
    ga_ = rng.integers(0, BPC, NIT)
    gb_ = rng.integers(0, BPC, NIT)
    ia_ = rng.integers(0, NCORES, NIT)
    ib_ = rng.integers(0, NCORES, NIT)
    for ga, gb, ia, ib in zip(ga_, gb_, ia_, ib_):
        if ga == gb:
            continue
        old = cost[ga] + cost[gb]
        grp[ga][ia], grp[gb][ib] = grp[gb][ib], grp[ga][ia]
        ca, cb = gcost(grp[ga]), gcost(grp[gb])
        if ca + cb <= old:
            cost[ga], cost[gb] = ca, cb
        else:
            grp[ga][ia], grp[gb][ib] = grp[gb][ib], grp[ga][ia]
    block2core = np.empty(NB, np.int64)
    block2slot = np.empty(NB, np.int64)
    for g in range(BPC):
        for m in range(NCORES):
            block2core[grp[g][m]] = m
            block2slot[grp[g][m]] = g
    return block2core, block2slot


def _pads(m):
    """m scattered ascending int16 idxs for gather pad slots."""
    return np.sort((np.arange(m, dtype=np.int64) * 97 % HALF)).astype(np.int16)


def _build_in_b(node_0, aug_full, al_full, ar_full,
                block2core, block2slot, offs, src_s, dst_s, local_s, w_s,
                A, B, slot_order, ln_weight, ln_bias):
    NB = NCORES * BPC
    TL, TH, T64, T128 = _tile_counts(A, B)
    TT = T64 + T128
    node0_pad = np.zeros((NPAD, D), np.float32)
    node0_pad[:N] = node_0
    gbv = np.concatenate([ln_weight, ln_bias])[None, :]
    iota64_np = np.tile(np.arange(64, dtype=np.float16)[None, :], (128, 1))
    iota128_np = np.tile(np.arange(128, dtype=np.float16)[None, :], (128, 1))
    epsi_np = (EPS_FA * np.eye(128, dtype=np.float32)).astype(np.float16)
    al16 = al_full.astype(np.float16)
    ar16 = ar_full.astype(np.float16)
    w16 = w_s.astype(np.float16)
    in_b = []
    for k in range(NCORES):
        lo_vals, hi_vals = [], []
        dstl_arr = np.zeros((128, TT), np.float16)
        w_arr = np.zeros((128, TT), np.float16)
        al_arr = np.zeros((128, TT), np.float16)
        ar_arr = np.zeros((128, TT), np.float16)
        col64 = T128
        col128 = 0

        def put_cols(col, ntile, sl_list, sent, dsub, h):
            """Fill ntile columns starting at col from edge slices sl_list
            (concatenated, in order). dsub subtracted from dst_local;
            sent = pad sentinel; h picks the gather base half."""
            idxs = np.concatenate([np.arange(s.start, s.stop)
                                   for s in sl_list]) if sl_list else \
                np.zeros(0, np.int64)
            nv = len(idxs)
            cap = ntile * 128
            dv = np.full(cap, sent, np.float16)
            dv[:nv] = local_s[idxs] - dsub
            wv = np.zeros(cap, np.float16)
            wv[:nv] = w16[idxs]
            av = np.zeros(cap, np.float16)
            av[:nv] = al16[src_s[idxs]]
            rv = np.zeros(cap, np.float16)
            rv[:nv] = ar16[dst_s[idxs]]
            dstl_arr[:, col:col + ntile] = dv.reshape(ntile, 128).T
            w_arr[:, col:col + ntile] = wv.reshape(ntile, 128).T
            al_arr[:, col:col + ntile] = av.reshape(ntile, 128).T
            ar_arr[:, col:col + ntile] = rv.reshape(ntile, 128).T
            iv = _pads(cap)
            iv[:nv] = (src_s[idxs] - HALF * h).astype(np.int16)
            return iv

        for i in range(BPC):
            for h in range(2):
                ki = ((k * BPC + int(slot_order[i])) * 2 + h) * 2
                s0, s1, s2 = offs[ki], offs[ki + 1], offs[ki + 2]
                n0, n1 = s1 - s0, s2 - s1
                a, b = int(A[i, h]), int(B[i, h])
                m0 = min(128 * a, n0)
                r0 = n0 - m0
                stk = min(128 - r0, n1)
                coll = lo_vals if h == 0 else hi_vals
                # W64 @0 tiles (bin0 head)
                iv = put_cols(col64, a, [slice(s0, s0 + m0)] if a else [],
                              100.0, 0, h)
                if a:
                    coll.append(iv)
                # straddle (bin0 tail + bin1 head)
                iv = put_cols(col128, 1,
                              [slice(s0 + m0, s1), slice(s1, s1 + stk)],
                              200.0, 0, h)
                coll.append(iv)
                # W64 @64 tiles (bin1 tail)
                iv = put_cols(col64 + a, b,
                              [slice(s1 + stk, s2)] if b else [], 100.0, 64,
                              h)
                if b:
                    coll.append(iv)
                col64 += a + b
                col128 += 1
        blocks_k = np.array([np.where((block2core == k) & (block2slot == i))[0][0]
                             for i in range(BPC)])
        node0_k = node0_pad.reshape(NB, 128, D)[blocks_k].reshape(NSH, D)
        in_b.append({
            "aug": aug_full,
            "idx_lo": _pack_gather_idxs(np.concatenate(lo_vals), TL),
            "idx_hi": _pack_gather_idxs(np.concatenate(hi_vals), TH),
            "dstl": dstl_arr,
            "wgt": w_arr,
            "alv": al_arr,
            "arv": ar_arr,
            "node0_sh": node0_k.astype(np.float16),
            "gb": gbv,
            "iota64_in": iota64_np,
            "iota128_in": iota128_np,
            "epsi_in": epsi_np,
        })
        _cache.setdefault("blocks_by_core", {})[k] = blocks_k
    return in_b


def kernel(node, node_0, edge_index, edge_attr, batch_ptr,
           att_l, att_r, ln_weight, ln_bias):
    node = np.asarray(node, np.float32)
    node_0 = np.asarray(node_0, np.float32)
    src = np.asarray(edge_index[0], np.int64)
    dst = np.asarray(edge_index[1], np.int64)
    w = np.asarray(edge_attr, np.float32)
    att_l = np.asarray(att_l, np.float32)
    att_r = np.asarray(att_r, np.float32)
    ln_weight = np.asarray(ln_weight, np.float32)
    ln_bias = np.asarray(ln_bias, np.float32)

    (block2core, block2slot, offs, src_s, dst_s, local_s, w_s,
     A, B, slot_order) = _host_shard(src, dst, w)

    gb_identity = bool(np.all(ln_weight == 1.0) and np.all(ln_bias == 0.0))
    sig = (A.tobytes(), B.tobytes(), gb_identity)
    if "A" not in _cache:
        _cache["A"] = _build_phase_a()
    if ("B", sig) not in _cache:
        _cache[("B", sig)] = _build_phase_b(A, B, gb_identity)
    nc_a = _cache["A"]
    nc_b = _cache[("B", sig)]

    # ---- phase A ----
    node_pad = np.zeros((NPAD, D), np.float32)
    node_pad[:N] = node
    att = np.stack([att_l, att_r])
    in_a = [{"node_sh": node_pad[k * NSH:(k + 1) * NSH], "att": att}
            for k in range(NCORES)]
    res_a = run_bass_kernel_spmd(nc_a, in_a, list(range(NCORES)),
                                 **_cache.get("runkw", {}))
    aug_full = np.concatenate([res_a.results[k]["aug_sh"]
                               for k in range(NCORES)])
    al_full = np.concatenate(
        [res_a.results[k]["alr_sh"][:, 0:BPC].T.reshape(NSH)
         for k in range(NCORES)])
    ar_full = np.concatenate(
        [res_a.results[k]["alr_sh"][:, BPC:2 * BPC].T.reshape(NSH)
         for k in range(NCORES)])
    t_a = res_a.exec_time_ns

    # ---- phase B ----
    in_b = _build_in_b(node_0, aug_full, al_full, ar_full,
                       block2core, block2slot, offs, src_s, dst_s, local_s,
                       w_s, A, B, slot_order, ln_weight, ln_bias)
    res_b = run_bass_kernel_spmd(nc_b, in_b, list(range(NCORES)),
                                 **_cache.get("runkw", {}))
    NB = NCORES * BPC
    out = np.empty((NB, 128, D), np.float32)
    for k in range(NCORES):
        out[_cache["blocks_by_core"][k]] = \
            res_b.results[k]["out_sh"].astype(np.float32).reshape(BPC, 128, D)
    out = out.reshape(NPAD, D)
    t_b = res_b.exec_time_ns
    _cache["t_a_ns"] = t_a
    _cache["t_b_ns"] = t_b
    _cache["res_a"] = res_a
    _cache["res_b"] = res_b
    if t_a is not None and t_b is not None:
        _cache["last_exec_ns"] = t_a + t_b
    return out[:N]
